# revision 111
# baseline (speedup 1.0000x reference)
"""MPNN-GGNN forward on 8 Trainium2 NeuronCores.

Data-parallel over the batch: 8 graphs per core. All weights replicated.
Per-core Bass/Tile kernel computes 4 message-passing + GRU steps and the
gated readout entirely on-chip; f32r (full fp32 bits, reduced-precision
multiplier) matmuls at full PE rate. The small-magnitude gh GEMM
(h @ Whh^T) and GRU bias rows run as fp8e4 DoubleRow matmuls (2 K-chunks
per instruction at 0.5 cycles/row = 4x f32r); the error-sensitive
message path (proj/agg/gi, preact sigma ~50) must stay f32r (measured:
fp8 there = 0.07 rel err, bf16 = 0.026, vs the 2e-2 gate).

Schedule notes (PE is in-order; head-of-line blocking is the enemy):
  - h-transposes run one graph behind their update chain
  - fp8-DR GRU matmuls are emitted before the mT-dependent f32r ones
  - last step runs fb1's graphs first; fb1's readout L0..L2 is injected
    into the final GRU iteration, and readout layers zipper between fbs
  - per-step node_mask multiply dropped (masked inputs make it redundant
    until the readout's mask_row)

Layout conventions per core (G = 8 graphs, N = 128 nodes, H = MSG = 512):
  h_all  [128(node), G, 512]  fp32   node-major hidden state
  hT_fb[fb] [128(feat), 4, 512]  h^T split per readout block so fb0's
         readout only depends on graphs 0-3
  hT8_all[128, G, HC, 128] fp8   h^T chunk-pairs for DoubleRow gh
  mask_sb[128(w),   G, L, 128(v)]    (e^T == l+1) one-hot adjacency
  matmul convention: out[i,j] = sum_k lhsT[k,i] * rhs[k,j]
"""

import numpy as np

import concourse.mybir as mybir
import concourse.tile as tile
from concourse import bacc
from concourse.bass_utils import run_bass_kernel_spmd

# problem constants (hardcoded per contract)
B, N, F_IN = 64, 128, 128
H, MSG, L = 512, 512, 4
NSTEP = 4
TARGET = 12
NCORES = 8
G = B // NCORES          # graphs per core
HC = H // 128            # h chunks
MC = MSG // 128          # msg chunks
FB = 2                   # readout free blocks (4 graphs x 128 nodes each)
GPB = G // FB

f32 = mybir.dt.float32
f32r = mybir.dt.float32r
f8 = mybir.dt.float8e4
AF = mybir.ActivationFunctionType
ALU = mybir.AluOpType
AX = mybir.AxisListType
DR = mybir.MatmulPerfMode.DoubleRow
GSC = 16.0  # gate-preact PSUM scale: wih/whh/biases premultiplied by 16
# Direct-mT aggregation is dead: hw forbids mixing f32r with 8/16-bit
# matmul inputs, f32r x f32r at free=128 pays the 4x narrow-moving-dim
# penalty, and a bf16 P loses too much precision (rel err 0.026 measured).
# The 0/1 mask still ships as fp8 (4x less serial prologue DMA) and is
# expanded to f32r on the idle Pool engine.
MASK8_DMA = True

_CACHE = {}


def _build(nreps=1):
    nc = bacc.Bacc("TRN2", target_bir_lowering=False)

    # ---- DRAM I/O ----
    # boot: hT0 for graph 0 + A[l=0..1, hc=0] packed in one early DMA
    d_boot = nc.dram_tensor("boot", [128, N + 2 * MSG], f32r,
                            kind="ExternalInput")
    d_h0 = nc.dram_tensor("h0", [N, G, F_IN], f32r, kind="ExternalInput")
    d_hT0 = nc.dram_tensor("hT0", [F_IN, G, N], f32r, kind="ExternalInput")
    d_mask = nc.dram_tensor("mask", [N, G, L, N],
                            f8 if MASK8_DMA else f32r, kind="ExternalInput")
    d_A = nc.dram_tensor("A", [128, L, HC, MSG], f32r, kind="ExternalInput")
    d_wih = nc.dram_tensor("wih", [128, MC, 3 * H], f32r, kind="ExternalInput")
    d_whh8 = nc.dram_tensor("whh8", [128, HC, 3 * H], f8, kind="ExternalInput")
    CR = N + TARGET  # packed const rows: ones|ones12
    d_crows = nc.dram_tensor("crows", [1, CR], f32r, kind="ExternalInput")
    # fp8 GRU bias rows (x16), block 1 all-zero (DoubleRow zero-pair partner)
    d_crows8 = nc.dram_tensor("crows8", [1, 2, 4 * H], f8, kind="ExternalInput")
    d_ones8 = nc.dram_tensor("ones8", [1, 2, N], f8, kind="ExternalInput")
    d_identcol = nc.dram_tensor("identcol", [128, 129], f32r, kind="ExternalInput")
    d_rowb = nc.dram_tensor("rowb", [128, 17, 128], f32r, kind="ExternalInput")
    d_row3 = nc.dram_tensor("row3", [128, 2, TARGET], f32r, kind="ExternalInput")
    d_robias = nc.dram_tensor("robias", [128, 8], f32, kind="ExternalInput")
    d_rob12 = nc.dram_tensor("rob12", [TARGET, 2], f32, kind="ExternalInput")
    d_out = nc.dram_tensor("out", [TARGET, G], f32, kind="ExternalOutput")

    with tile.TileContext(nc) as tc:
        with tc.tile_pool(name="st", bufs=1) as st, \
             tc.tile_pool(name="state", bufs=1) as stt, \
             tc.tile_pool(name="wk", bufs=2) as wk, \
             tc.tile_pool(name="ps", bufs=1, space="PSUM") as ps:

            # ---- static loads, in consumption order ----
            # first PE ops need hT0 + identcol + A[l, hc=0] + mask; then the
            # step-0 GRU needs wih/whh8/bias rows; the rest can trickle in
            boot_t = st.tile([128, N + 2 * MSG], f32r, tag="boot")
            nc.sync.dma_start(boot_t[:], d_boot[:])
            boot_hT0g0 = boot_t[:, 0:N]
            boot_A0 = [boot_t[:, N:N + MSG], boot_t[:, N + MSG:N + 2 * MSG]]
            hT0_sb = st.tile([F_IN, G, N], f32r, tag="hT0")
            A_sb = st.tile([128, L, HC, MSG], f32r, tag="A")
            nc.sync.dma_start(A_sb[:, 2, 0, :], d_A[:, 2, 0, :])
            nc.sync.dma_start(A_sb[:, 3, 0, :], d_A[:, 3, 0, :])
            nc.sync.dma_start(hT0_sb[:], d_hT0[:])
            nc.sync.dma_start(A_sb[:, 0, 0, :], d_A[:, 0, 0, :])
            nc.sync.dma_start(A_sb[:, 1, 0, :], d_A[:, 1, 0, :])
            identcol_t = st.tile([128, 129], f32r, tag="identcol")
            nc.sync.dma_start(identcol_t[:], d_identcol[:])
            ident_sb = identcol_t[:, 0:128]
            onescol_sb = identcol_t[:, 128:129]
            mask_sb = st.tile([N, G, L, N], f32r, tag="mask")
            if MASK8_DMA:
                mask8st = st.tile([N, 2, L, N], f8, tag="mask8st")
                for g_ in range(G):
                    sl8 = g_ % 2
                    nc.sync.dma_start(mask8st[:, sl8, :, :],
                                      d_mask[:, g_, :, :])
                    nc.gpsimd.tensor_scalar_mul(mask_sb[:, g_, :, :],
                                                mask8st[:, sl8, :, :], 1.0)
            else:
                for g_ in range(G):
                    nc.sync.dma_start(mask_sb[:, g_, :, :],
                                      d_mask[:, g_, :, :])
            wih_sb = st.tile([128, MC, 3 * H], f32r, tag="wih")
            for c in range(MC):
                nc.sync.dma_start(wih_sb[:, c, :], d_wih[:, c, :])
            whh8_sb = st.tile([128, HC, 3 * H], f8, tag="whh8")
            nc.sync.dma_start(whh8_sb[:], d_whh8[:])
            crows8_t = st.tile([1, 2, 4 * H], f8, tag="crows8")
            nc.sync.dma_start(crows8_t[:], d_crows8[:])
            ones8_t = st.tile([1, 2, N], f8, tag="ones8")
            nc.sync.dma_start(ones8_t[:], d_ones8[:])
            h_all = stt.tile([N, G, H], f32r, tag="h_all")
            nc.sync.dma_start(h_all[:, :, 0:F_IN], d_h0[:])
            nc.gpsimd.memset(h_all[:, :, F_IN:H].bitcast(f32), 0.0)
            hT_fb = [stt.tile([128, GPB, H], f32r, tag=f"hT_fb{fb_}",
                              name=f"hT_fb{fb_}") for fb_ in range(FB)]
            hT8_all = stt.tile([128, G, HC, 128], f8, tag="hT8_all")
            hT08 = stt.tile([128, G, 2, 128], f8, tag="hT08")
            nc.gpsimd.memset(hT08[:, :, 1, :], 0.0)
            nc.gpsimd.tensor_scalar_mul(hT08[:, :, 0, :], hT0_sb[:], 1.0)
            crows_t = st.tile([1, CR], f32r, tag="crows")
            nc.sync.dma_start(crows_t[:], d_crows[:])
            ones12_sb = crows_t[:, N:N + TARGET]
            for hc_ in range(1, HC):
                for l_ in range(L):
                    nc.sync.dma_start(A_sb[:, l_, hc_, :], d_A[:, l_, hc_, :])

            rowb_t = st.tile([128, 17, 128], f32r, tag="rowb")
            nc.sync.dma_start(rowb_t[:], d_rowb[:])
            r1w0_sb = rowb_t[:, 0:5, :]
            r1w1_sb = rowb_t[:, 5:7, :]
            r1w2_sb = rowb_t[:, 7:9, :]
            r2w0_sb = rowb_t[:, 9:13, :]
            r2w1_sb = rowb_t[:, 13:15, :]
            r2w2_sb = rowb_t[:, 15:17, :]
            row3_t = st.tile([128, 2, TARGET], f32r, tag="row3")
            nc.sync.dma_start(row3_t[:], d_row3[:])
            r1w3_sb = row3_t[:, 0, :]
            r2w3_sb = row3_t[:, 1, :]
            robias_t = st.tile([128, 8], f32, tag="robias")
            nc.sync.dma_start(robias_t[:], d_robias[:])
            r1b0_sb = robias_t[:, 0:1]
            r1b1_sb = robias_t[:, 1:3]
            r1b2_sb = robias_t[:, 3:4]
            r2b0_sb = robias_t[:, 4:5]
            r2b1_sb = robias_t[:, 5:7]
            r2b2_sb = robias_t[:, 7:8]
            rob12_t = st.tile([TARGET, 2], f32, tag="rob12")
            nc.sync.dma_start(rob12_t[:], d_rob12[:])
            r1b3_sb = rob12_t[:, 0:1]
            r2b3_sb = rob12_t[:, 1:2]


            for _rep in range(nreps):
                # NOTE: the reference's per-step node_mask multiply is
                # dropped: masked inputs guarantee no edges touch virtual
                # nodes, their per-node GRU lanes never mix into real nodes,
                # and the readout re-applies mask_row.

                # ---- message passing loop ----
                def hT_chunk(s, g, hc):
                    if s == 0:
                        assert hc == 0
                        return hT0_sb[:, g, :]
                    return hT_fb[g // GPB][:, g % GPB,
                                           hc * 128:(hc + 1) * 128]

                pending_htr = []  # delayed h-transposes (PE in-order relief)

                def h_transpose(s, g):
                    # h^T for next step / readout; one graph behind the GRU
                    # so the PE never head-of-line blocks on g's update chain
                    tp2 = ps.tile([128, H], f32r, tag="pMT", bufs=2,
                                  name=f"tp2_{s}_{g}")
                    for c in range(HC):
                        nc.tensor.transpose(tp2[:, c * 128:(c + 1) * 128],
                                            h_all[:, g, c * 128:(c + 1) * 128],
                                            ident_sb[:])
                    hTg = hT_fb[g // GPB][:, g % GPB, :]
                    if g % 2 == 0:
                        nc.vector.tensor_copy(hTg, tp2[:])
                    else:
                        nc.scalar.copy(hTg, tp2[:])
                    if s < NSTEP - 1:  # fp8 h^T for next step's gh GEMM
                        # (gpsimd cannot read PSUM -> source the SBUF copy)
                        nc.gpsimd.tensor_scalar_mul(hT8_all[:, g, :, :],
                                                    hTg, 1.0)

                # ---- readout (layer-major over 4 independent chains) ----
                out_sb = st.tile([TARGET, G], f32, tag="out_sb")
                nfree = GPB * N  # 512
                r1_ws = [[r1w0_sb[:, kc, :] for kc in range(5)],
                         [r1w1_sb[:, oc, :] for oc in range(2)],
                         [r1w2_sb[:, kc, :] for kc in range(2)],
                         r1w3_sb[:]]
                r1_bs = [r1b0_sb[:],
                         [r1b1_sb[:, oc:oc + 1] for oc in range(2)],
                         r1b2_sb[:]]
                r2_ws = [[r2w0_sb[:, kc, :] for kc in range(4)],
                         [r2w1_sb[:, oc, :] for oc in range(2)],
                         [r2w2_sb[:, kc, :] for kc in range(2)],
                         r2w3_sb[:]]
                r2_bs = [r2b0_sb[:],
                         [r2b1_sb[:, oc:oc + 1] for oc in range(2)],
                         r2b2_sb[:]]
                chains = []
                for fb in range(FB):
                    gsl = slice(fb * GPB, (fb + 1) * GPB)
                    h_in_chunks = [hT_fb[fb][:, :, kc * 128:(kc + 1) * 128]
                                   for kc in range(HC)]
                    chains.append(dict(fb=fb, w="g", ws=r1_ws, bs=r1_bs,
                                       ins=h_in_chunks + [hT0_sb[:, gsl, :]]))
                    chains.append(dict(fb=fb, w="v", ws=r2_ws, bs=r2_bs,
                                       ins=h_in_chunks))
                relueng = [None, nc.vector, None, nc.vector]

                def relu_from(dst, src, bias, ci):
                    eng = relueng[ci]
                    if eng is None:
                        nc.scalar.activation(dst, src, AF.Relu, bias=bias)
                    else:
                        eng.tensor_scalar(dst, src, bias, 0.0,
                                          op0=ALU.add, op1=ALU.max)

                def ro_l0(ch, ci):
                    key = f"{ch['w']}{ch['fb']}"
                    p = ps.tile([128, nfree], f32,
                                tag="pP" if ci % 2 == 0 else "pG2", bufs=2,
                                name=f"rop0_{key}")
                    for i, (wap, rhs) in enumerate(zip(ch["ws"][0], ch["ins"])):
                        nc.tensor.matmul(p[:], wap, rhs, start=(i == 0),
                                         stop=(i == len(ch["ins"]) - 1))
                    a1 = wk.tile([128, nfree], f32r, tag="P", bufs=8,
                                 name=f"roa1_{key}")
                    relu_from(a1[:], p[:], ch["bs"][0], ci)
                    ch["a1"] = a1

                # fb=0 L0 first: it only needs graphs 0-3, so the PE can run
                # it while graph 7's update chain is still in flight; the
                # last h-transpose is flushed in between
                def ro_l1(fb):
                    for ci0, ch in enumerate(chains[2 * fb:2 * fb + 2]):
                        ci = 2 * fb + ci0
                        key = f"{ch['w']}{ch['fb']}"
                        ch["a2"] = []
                        for oc in range(2):
                            p2 = ps.tile([128, nfree], f32,
                                         tag="pP" if oc == 0 else "pG2",
                                         bufs=2, name=f"rop1_{key}_{oc}")
                            nc.tensor.matmul(p2[:], ch["ws"][1][oc],
                                             ch["a1"][:],
                                             start=True, stop=True)
                            t = wk.tile([128, nfree], f32r, tag="P", bufs=8,
                                        name=f"roa2_{key}_{oc}")
                            relu_from(t[:], p2[:], ch["bs"][1][oc],
                                      (ci + oc) % 4)
                            ch["a2"].append(t)

                def ro_l2(fb):
                    for ci0, ch in enumerate(chains[2 * fb:2 * fb + 2]):
                        ci = 2 * fb + ci0
                        key = f"{ch['w']}{ch['fb']}"
                        p3 = ps.tile([128, nfree], f32,
                                      tag="pP" if ci0 == 0 else "pG2",
                                      bufs=2, name=f"rop2_{key}")
                        for kc in range(2):
                            nc.tensor.matmul(p3[:], ch["ws"][2][kc],
                                             ch["a2"][kc][:],
                                             start=(kc == 0), stop=(kc == 1))
                        a3 = wk.tile([128, nfree], f32r, tag="P", bufs=8,
                                     name=f"roa3_{key}")
                        relu_from(a3[:], p3[:], ch["bs"][2], ci)
                        ch["a3"] = a3

                def ro_l3(fb):
                    # v-chain first: its PSUM feeds the DVE vm op while the
                    # g-chain's L3 + sigmoid still run
                    for ch in chains[2 * fb:2 * fb + 2][::-1]:
                        key = f"{ch['w']}{ch['fb']}"
                        p4 = ps.tile([TARGET, nfree], f32, tag="pGN", bufs=2,
                                     name=f"rop3_{key}")
                        nc.tensor.matmul(p4[:], ch["ws"][3], ch["a3"][:],
                                         start=True, stop=True)
                        ch["p4"] = p4

                def ro_finals(fb):
                    # finals: sum_v gate*val*mask per graph
                    fsl = slice(fb * nfree, (fb + 1) * nfree)
                    chg, chv = chains[2 * fb:2 * fb + 2]
                    # val*mask fused: (p4 + b) * mb, straight from PSUM
                    vm = wk.tile([TARGET, nfree], f32, tag="z", bufs=2,
                                 name=f"vm_{fb}")
                    nc.vector.scalar_tensor_tensor(
                        vm[:], chv["p4"][:], r2b3_sb[:], mb_sb[:, fsl],
                        op0=ALU.add, op1=ALU.mult)
                    gate_s = wk.tile([TARGET, nfree], f32, tag="r", bufs=2,
                                     name=f"gate_{fb}")
                    nc.scalar.activation(gate_s[:], chg["p4"][:], AF.Sigmoid,
                                         bias=r1b3_sb[:])
                    pr2 = wk.tile([TARGET, GPB, N], f32, tag="t1", bufs=2,
                                  name=f"pr2_{fb}")
                    nc.vector.scalar_tensor_tensor(
                        pr2[:], gate_s[:], 1.0, vm[:],
                        op0=ALU.mult, op1=ALU.mult)
                    nc.vector.tensor_reduce(
                        out_sb[:, fb * GPB:(fb + 1) * GPB], pr2[:],
                        axis=AX.X, op=ALU.add)

                for s in range(NSTEP):
                    hcs = [0] if s == 0 else list(range(HC))
                    # -- phase 1 (all graphs): projections + aggregation -> mT --
                    def projections(g):
                        P_sb = []
                        cpeng = [nc.vector.tensor_copy, nc.scalar.copy,
                                 nc.scalar.copy, nc.scalar.copy]
                        if s == 0:
                            # step-0 phase 1 is copy-bound on Act: rebalance
                            # one PSUM->SBUF copy onto DVE
                            cpeng[2] = nc.vector.tensor_copy
                        for l in range(L):
                            # step 0: spread projection PSUM across the pP
                            # and (GRU-idle) pG2 pools -> 4 effective bufs,
                            # so short s0 projections don't wait on copies
                            ptag = "pP" if l % 2 == 0 else "pG2"
                            pp = ps.tile([128, MSG], f32, tag=ptag, bufs=2,
                                         name=f"pp_{s}_{g}_{l}")
                            if s == 0 and g == 0 and l < 2:
                                # sourced from the packed boot DMA: the very
                                # first matmuls wait on one small transfer
                                nc.tensor.matmul(pp[:], boot_hT0g0,
                                                 boot_A0[l],
                                                 start=True, stop=True)
                                psb = wk.tile([128, MSG], f32r, tag="P",
                                              bufs=8, name=f"psb_{s}_{g}_{l}")
                                cpeng[l](psb[:], pp[:])
                                P_sb.append(psb)
                                continue
                            for i, hc in enumerate(hcs):
                                nc.tensor.matmul(pp[:], hT_chunk(s, g, hc),
                                                 A_sb[:, l, hc, :],
                                                 start=(i == 0),
                                                 stop=(i == len(hcs) - 1))
                            psb = wk.tile([128, MSG], f32r, tag="P", bufs=8,
                                          name=f"psb_{s}_{g}_{l}")
                            cpeng[l](psb[:], pp[:])
                            P_sb.append(psb)
                        return P_sb

                    def agg_m(g, P_sb):
                        mp = ps.tile([128, MSG], f32, tag="pMT", bufs=2,
                                     name=f"mp_{s}_{g}")
                        for l in range(L):
                            nc.tensor.matmul(mp[:], mask_sb[:, g, l, :], P_sb[l][:],
                                             start=(l == 0), stop=(l == L - 1))
                        m_sb = wk.tile([128, MSG], f32r, tag="m", bufs=2,
                                       name=f"m_{s}_{g}")
                        nc.vector.tensor_copy(m_sb[:], mp[:])
                        return m_sb

                    def mT_from(g, m_sb):
                        tp = ps.tile([128, MSG], f32r, tag="pMT", bufs=2,
                                     name=f"tp_{s}_{g}")
                        for c in range(MC):
                            nc.tensor.transpose(tp[:, c * 128:(c + 1) * 128],
                                                m_sb[:, c * 128:(c + 1) * 128],
                                                ident_sb[:])
                        mT_sb = wk.tile([128, MSG], f32r, tag="mT", bufs=6,
                                        name=f"mT_{s}_{g}")
                        nc.scalar.copy(mT_sb[:], tp[:])
                        return mT_sb

                    # last step runs fb1's graphs first so their h^T land
                    # early for the (fb1-first) readout
                    gorder = (list(range(G)) if s < NSTEP - 1
                              else list(range(GPB, G)) + list(range(GPB)))
                    mT_tiles = {}
                    for gp in range(G // 2):
                        g0, g1 = gorder[2 * gp], gorder[2 * gp + 1]
                        Ps0 = projections(g0)
                        if gp == 0 and pending_htr:
                            h_transpose(*pending_htr.pop())
                        Ps1 = projections(g1)
                        mT_tiles[g0] = mT_from(g0, agg_m(g0, Ps0))
                        mT_tiles[g1] = mT_from(g1, agg_m(g1, Ps1))
                    if s == 0:
                        # read mask + its 12-row broadcast; emitted here to
                        # keep it off the startup critical path (only the
                        # readout finals consume these)
                        mask_row = st.tile([1, G * N], f32r, tag="mask_row")
                        mb_sb = st.tile([TARGET, G * N], f32, tag="mb_sb")
                        for fb in range(FB):
                            fsl = slice(fb * GPB * N, (fb + 1) * GPB * N)
                            colsum = ps.tile([1, GPB * N], f32, tag="pMT",
                                             bufs=2, name=f"colsum{fb}")
                            nc.tensor.matmul(
                                colsum[:], onescol_sb[:],
                                hT0_sb[:, fb * GPB:(fb + 1) * GPB, :],
                                start=True, stop=True)
                            nc.vector.tensor_scalar(mask_row[:, fsl],
                                                    colsum[:], 0.0, None,
                                                    op0=ALU.not_equal)
                            mbp = ps.tile([TARGET, GPB * N], f32, tag="pMT",
                                          bufs=2, name=f"mbp{fb}")
                            nc.tensor.matmul(mbp[:], ones12_sb[:],
                                             mask_row[:, fsl],
                                             start=True, stop=True)
                            nc.scalar.copy(mb_sb[:, fsl], mbp[:])
                    # -- phase 2 (all graphs): GRU GEMMs + gates --
                    def gh8_pairs(g, sl):
                        # fp8 DoubleRow h @ Whh chunk-pairs (zero-pair at s=0)
                        if s == 0:
                            return [(hT08[:, g, :, :], whh8_sb[:, 0:2, sl])]
                        return [(hT8_all[:, g, c:c + 2, :],
                                 whh8_sb[:, c:c + 2, sl]) for c in (0, 2)]

                    def gate_mms(o, dr_mms, f32_mms):
                        # fp8-DR insts first: they depend only on hT8/bias
                        # rows, so the PE can start them before the mT copy
                        # for this graph has landed
                        ntot = len(f32_mms) + len(dr_mms)
                        i = 0
                        for lh, rh in dr_mms:
                            nc.tensor.matmul(o, lh, rh, start=(i == 0),
                                             stop=(i == ntot - 1),
                                             perf_mode=DR)
                            i += 1
                        for lh, rh in f32_mms:
                            nc.tensor.matmul(o, lh, rh, start=(i == 0),
                                             stop=(i == ntot - 1))
                            i += 1

                    for gi_, g in enumerate(gorder):
                        mT_sb = mT_tiles[g]
                        # 4) GRU gate GEMMs: gi/mT in f32r, gh + biases in
                        # fp8 DoubleRow; PSUM carries GSC x the true preacts
                        rzp = [ps.tile([128, H], f32, tag="pG2", bufs=2,
                                       name=f"g{'rz'[h_]}_{s}_{g}")
                               for h_ in range(2)]
                        gin = ps.tile([128, H], f32, tag="pGN", bufs=2,
                                      name=f"gin_{s}_{g}")
                        ghn = ps.tile([128, H], f32, tag="pGN", bufs=2,
                                      name=f"ghn_{s}_{g}")
                        gate_mms(
                            ghn[:],
                            gh8_pairs(g, slice(2 * H, 3 * H))
                            + [(ones8_t[:], crows8_t[:, :, 3 * H:4 * H])],
                            [])
                        for half in range(2):
                            sl = slice(half * H, (half + 1) * H)
                            gate_mms(
                                rzp[half][:],
                                gh8_pairs(g, sl)
                                + [(ones8_t[:], crows8_t[:, :, sl])],
                                [(mT_sb[:, c * 128:(c + 1) * 128],
                                  wih_sb[:, c, sl]) for c in range(MC)])
                        gate_mms(
                            gin[:],
                            [(ones8_t[:], crows8_t[:, :, 2 * H:3 * H])],
                            [(mT_sb[:, c * 128:(c + 1) * 128],
                              wih_sb[:, c, 2 * H:3 * H]) for c in range(MC)])
                        # 5) gate nonlinearities + state update as
                        # h' = (1-z)*n + z*h: z*h runs before tanh lands,
                        # so only 2 DVE ops trail the tanh
                        r_sb = wk.tile([128, H], f32, tag="r", bufs=2,
                                       name=f"r_{s}_{g}")
                        nc.scalar.activation(r_sb[:], rzp[0][:], AF.Sigmoid,
                                             scale=1.0 / GSC)
                        z_sb = wk.tile([128, H], f32, tag="z", bufs=2,
                                       name=f"z_{s}_{g}")
                        nc.scalar.activation(z_sb[:], rzp[1][:],
                                             AF.Sigmoid, scale=1.0 / GSC)
                        # u = 1-z via negated-scale sigmoid
                        u_sb = wk.tile([128, H], f32, tag="u", bufs=2,
                                       name=f"u_{s}_{g}")
                        nc.scalar.activation(u_sb[:], rzp[1][:], AF.Sigmoid,
                                             scale=-1.0 / GSC)
                        rhn = wk.tile([128, H], f32, tag="t1", bufs=2,
                                      name=f"rhn_{s}_{g}")
                        nc.vector.tensor_mul(rhn[:], r_sb[:], ghn[:])
                        zh = wk.tile([128, H], f32, tag="zh", bufs=2,
                                     name=f"zh_{s}_{g}")
                        nc.vector.tensor_mul(zh[:], z_sb[:], h_all[:, g, :])
                        npre = wk.tile([128, H], f32, tag="t2", bufs=2,
                                       name=f"npre_{s}_{g}")
                        nc.vector.tensor_add(npre[:], rhn[:], gin[:])
                        n_sb = wk.tile([128, H], f32, tag="n", bufs=2,
                                       name=f"n_{s}_{g}")
                        nc.scalar.activation(n_sb[:], npre[:], AF.Tanh,
                                             scale=1.0 / GSC)
                        un = wk.tile([128, H], f32, tag="t1", bufs=2,
                                     name=f"un_{s}_{g}")
                        nc.vector.tensor_mul(un[:], u_sb[:], n_sb[:])
                        nc.vector.tensor_add(h_all[:, g, :], un[:], zh[:])
                        pending_htr.append((s, g))
                        if s == NSTEP - 1 and gi_ == G - 1:
                            # fb1's h^T is complete (graphs 4-7 ran first +
                            # their transposes flushed): overlap its readout
                            # L0..L2 with the remaining fb0 GRU work
                            ro_l0(chains[2], 2)
                            ro_l0(chains[3], 3)
                            ro_l1(1)
                            ro_l2(1)
                        if gi_ >= 1:
                            h_transpose(*pending_htr.pop(0))

                # zippered, fb1 first (its graphs ran first in step 3):
                # fb1's L0 covers the last fb0 graph's update chain, then
                # layers alternate fbs so each fb's relu latency hides
                # under the other fb's GEMMs
                while pending_htr:
                    h_transpose(*pending_htr.pop(0))
                ro_l0(chains[0], 0)
                ro_l0(chains[1], 1)
                ro_l3(1)
                ro_l1(0)
                ro_finals(1)
                ro_l2(0)
                ro_l3(0)
                ro_finals(0)
                nc.sync.dma_start(d_out[:], out_sb[:])


    nc.compile()
    return nc


def _prep_core_inputs(core, g_, h_in, e, A, gru_Wih, gru_Whh, gru_bih,
                      gru_bhh, r1_Ws, r1_bs, r2_Ws, r2_bs):
    cs = slice(core * G, (core + 1) * G)
    f = np.float32
    h0 = np.asarray(h_in[cs], f)
    hT0 = np.ascontiguousarray(h_in[cs].transpose(2, 0, 1))  # [F, G, N]
    labels = np.arange(1, L + 1, dtype=f)
    # mask[w, g, l, v] = (e[g, v, w] == l+1)
    e_c = e[cs]  # [G, V, W]
    oh = (e_c[:, None, :, :] == labels[None, :, None, None]).astype(f)  # [G,L,V,W]
    mask = np.ascontiguousarray(oh.transpose(3, 0, 1, 2))  # [W, G, L, V]
    if MASK8_DMA:
        import ml_dtypes
        mask = mask.astype(ml_dtypes.float8_e4m3)
    boot = np.concatenate([hT0[:, 0, :], np.asarray(A[0, 0:128, :], f),
                           np.asarray(A[1, 0:128, :], f)], 1)
    return {
        "h0": np.ascontiguousarray(h0.transpose(1, 0, 2)),  # [N, G, F_IN]
        "hT0": hT0,
        "mask": mask,
        "boot": np.ascontiguousarray(boot),
    }


def _prep_shared_inputs(A, gru_Wih, gru_Whh, gru_bih, gru_bhh,
                        r1_Ws, r1_bs, r2_Ws, r2_bs):
    f = np.float32

    def chunk_rows(M, nch):  # [K, C] -> [128, nch, C] with K = nch*128
        K, C = M.shape
        assert K == nch * 128
        return np.ascontiguousarray(M.reshape(nch, 128, C).transpose(1, 0, 2))

    import ml_dtypes
    f8np = ml_dtypes.float8_e4m3
    GSC = np.float32(16.0)

    A_t = np.ascontiguousarray(
        A.reshape(L, HC, 128, MSG).transpose(2, 0, 1, 3))  # [128, L, HC, MSG]
    wih = chunk_rows(np.ascontiguousarray(gru_Wih.T) * GSC, MC)  # [128,MC,3H]
    whh8 = chunk_rows(np.ascontiguousarray(gru_Whh.T) * GSC, HC).astype(f8np)
    brz = (gru_bih + gru_bhh)[:2 * H].astype(f)
    bin_ = gru_bih[2 * H:].astype(f)
    bhn = gru_bhh[2 * H:].astype(f)
    crows8 = np.zeros((1, 2, 4 * H), f)
    crows8[0, 0, :] = np.concatenate([brz, bin_, bhn]) * GSC
    ones8 = np.ones((1, 2, N), f8np)

    # readout weights, transposed layout
    r1w0t = np.ascontiguousarray(r1_Ws[0].T)  # [2H, 128]
    r1w0 = np.zeros((128, 5, 128), f)
    for kc in range(4):
        r1w0[:, kc, :] = r1w0t[kc * 128:(kc + 1) * 128]
    r1w0[:, 4, :] = r1w0t[H:H + F_IN]  # h0 chunk (features 0:128 of h0 half)
    r1w1 = np.ascontiguousarray(r1_Ws[1].T.reshape(128, 2, 128))
    r1w2 = chunk_rows(np.ascontiguousarray(r1_Ws[2].T), 2)
    r1w3 = np.ascontiguousarray(r1_Ws[3].T)  # [128, 12]
    r2w0 = chunk_rows(np.ascontiguousarray(r2_Ws[0].T), 4)
    r2w1 = np.ascontiguousarray(r2_Ws[1].T.reshape(128, 2, 128))
    r2w2 = chunk_rows(np.ascontiguousarray(r2_Ws[2].T), 2)
    r2w3 = np.ascontiguousarray(r2_Ws[3].T)

    crows = np.concatenate([np.ones(N, f), np.ones(TARGET, f)])[None, :]
    identcol = np.concatenate([np.eye(128, dtype=f), np.ones((128, 1), f)], 1)
    rowb = np.concatenate([r1w0, r1w1, r1w2, r2w0, r2w1, r2w2], axis=1)
    row3 = np.stack([r1w3, r2w3], axis=1)
    robias = np.concatenate([
        r1_bs[0].reshape(-1, 1).astype(f),
        np.ascontiguousarray(r1_bs[1].reshape(2, 128).T),
        r1_bs[2].reshape(-1, 1).astype(f),
        r2_bs[0].reshape(-1, 1).astype(f),
        np.ascontiguousarray(r2_bs[1].reshape(2, 128).T),
        r2_bs[2].reshape(-1, 1).astype(f)], axis=1)
    rob12 = np.concatenate([r1_bs[3].reshape(-1, 1).astype(f),
                            r2_bs[3].reshape(-1, 1).astype(f)], axis=1)
    return {
        "A": A_t, "wih": wih, "whh8": np.ascontiguousarray(whh8),
        "crows": np.ascontiguousarray(crows),
        "crows8": crows8.astype(f8np),
        "ones8": ones8,
        "identcol": np.ascontiguousarray(identcol),
        "rowb": np.ascontiguousarray(rowb),
        "row3": np.ascontiguousarray(row3),
        "robias": np.ascontiguousarray(robias),
        "rob12": np.ascontiguousarray(rob12),
    }


def _get_nc(nreps=1):
    key = ("nc", nreps)
    if key not in _CACHE:
        _CACHE[key] = _build(nreps)
    return _CACHE[key]


def _run(in_maps, **kwargs):
    nc = _get_nc()
    return run_bass_kernel_spmd(nc, in_maps, core_ids=list(range(NCORES)),
                                **kwargs)


def make_in_maps(g, h_in, e, A, gru_Wih, gru_Whh, gru_bih, gru_bhh,
                 r1_W0, r1_b0, r1_W1, r1_b1, r1_W2, r1_b2, r1_W3, r1_b3,
                 r2_W0, r2_b0, r2_W1, r2_b1, r2_W2, r2_b2, r2_W3, r2_b3):
    r1_Ws, r1_bs = [r1_W0, r1_W1, r1_W2, r1_W3], [r1_b0, r1_b1, r1_b2, r1_b3]
    r2_Ws, r2_bs = [r2_W0, r2_W1, r2_W2, r2_W3], [r2_b0, r2_b1, r2_b2, r2_b3]
    arrs = {k: np.asarray(v, np.float32) for k, v in dict(
        g=g, h_in=h_in, e=e, A=A, gru_Wih=gru_Wih, gru_Whh=gru_Whh,
        gru_bih=gru_bih, gru_bhh=gru_bhh).items()}
    r1_Ws = [np.asarray(w, np.float32) for w in r1_Ws]
    r1_bs = [np.asarray(b, np.float32) for b in r1_bs]
    r2_Ws = [np.asarray(w, np.float32) for w in r2_Ws]
    r2_bs = [np.asarray(b, np.float32) for b in r2_bs]
    shared = _prep_shared_inputs(arrs["A"], arrs["gru_Wih"], arrs["gru_Whh"],
                                 arrs["gru_bih"], arrs["gru_bhh"],
                                 r1_Ws, r1_bs, r2_Ws, r2_bs)
    in_maps = []
    for core in range(NCORES):
        m = dict(shared)
        m.update(_prep_core_inputs(core, arrs["g"], arrs["h_in"], arrs["e"],
                                   arrs["A"], arrs["gru_Wih"], arrs["gru_Whh"],
                                   arrs["gru_bih"], arrs["gru_bhh"],
                                   r1_Ws, r1_bs, r2_Ws, r2_bs))
        in_maps.append(m)
    return in_maps


def kernel(**inputs):
    in_maps = make_in_maps(**inputs)
    res = _run(in_maps)
    out = np.zeros((B, TARGET), np.float32)
    for core in range(NCORES):
        out[core * G:(core + 1) * G] = res.results[core]["out"].T
    return out


if __name__ == "__main__":
    import reference
    inputs = {k: np.asarray(v) for k, v in reference.setup_inputs().items()}
    expected = np.asarray(reference.reference(**inputs))
    actual = kernel(**inputs)
    scale = np.abs(expected).max()
    err = np.abs(actual - expected).max() / scale
    print("Relative error:", err)



# revision 127
# speedup vs baseline: 1.0001x; 1.0001x over previous
"""MPNN-GGNN forward on 8 Trainium2 NeuronCores.

Data-parallel over the batch: 8 graphs per core. All weights replicated.
Per-core Bass/Tile kernel computes 4 message-passing + GRU steps and the
gated readout entirely on-chip; f32r (full fp32 bits, reduced-precision
multiplier) matmuls at full PE rate. The small-magnitude gh GEMM
(h @ Whh^T) and GRU bias rows run as fp8e4 DoubleRow matmuls (2 K-chunks
per instruction at 0.5 cycles/row = 4x f32r); the error-sensitive
message path (proj/agg/gi, preact sigma ~50) must stay f32r (measured:
fp8 there = 0.07 rel err, bf16 = 0.026, vs the 2e-2 gate).

Schedule notes (PE is in-order; head-of-line blocking is the enemy):
  - h-transposes run one graph behind their update chain
  - fp8-DR GRU matmuls are emitted before the mT-dependent f32r ones
  - last step runs fb1's graphs first; fb1's readout L0..L2 is injected
    into the final GRU iteration, and readout layers zipper between fbs
  - per-step node_mask multiply dropped (masked inputs make it redundant
    until the readout's mask_row)

Layout conventions per core (G = 8 graphs, N = 128 nodes, H = MSG = 512):
  h_all  [128(node), G, 512]  fp32   node-major hidden state
  hT_fb[fb] [128(feat), 4, 512]  h^T split per readout block so fb0's
         readout only depends on graphs 0-3
  hT8_all[128, G, HC, 128] fp8   h^T chunk-pairs for DoubleRow gh
  mask_sb[128(w),   G, L, 128(v)]    (e^T == l+1) one-hot adjacency
  matmul convention: out[i,j] = sum_k lhsT[k,i] * rhs[k,j]
"""

import numpy as np

import concourse.mybir as mybir
import concourse.tile as tile
from concourse import bacc
from concourse.bass_utils import run_bass_kernel_spmd

# problem constants (hardcoded per contract)
B, N, F_IN = 64, 128, 128
H, MSG, L = 512, 512, 4
NSTEP = 4
TARGET = 12
NCORES = 8
G = B // NCORES          # graphs per core
HC = H // 128            # h chunks
MC = MSG // 128          # msg chunks
FB = 2                   # readout free blocks (4 graphs x 128 nodes each)
GPB = G // FB

f32 = mybir.dt.float32
f32r = mybir.dt.float32r
f8 = mybir.dt.float8e4
AF = mybir.ActivationFunctionType
ALU = mybir.AluOpType
AX = mybir.AxisListType
DR = mybir.MatmulPerfMode.DoubleRow
GSC = 16.0  # gate-preact PSUM scale: wih/whh/biases premultiplied by 16
# Direct-mT aggregation is dead: hw forbids mixing f32r with 8/16-bit
# matmul inputs, f32r x f32r at free=128 pays the 4x narrow-moving-dim
# penalty, and a bf16 P loses too much precision (rel err 0.026 measured).
# The 0/1 mask still ships as fp8 (4x less serial prologue DMA) and is
# expanded to f32r on the idle Pool engine.
MASK8_DMA = True

_CACHE = {}


def _build(nreps=1):
    nc = bacc.Bacc("TRN2", target_bir_lowering=False)

    # ---- DRAM I/O ----
    # boot: hT0 for graph 0 + A[l=0..1, hc=0] packed in one early DMA
    d_boot = nc.dram_tensor("boot", [128, N + 2 * MSG], f32r,
                            kind="ExternalInput")
    d_h0 = nc.dram_tensor("h0", [N, G, F_IN], f32r, kind="ExternalInput")
    d_hT0 = nc.dram_tensor("hT0", [F_IN, G, N], f32r, kind="ExternalInput")
    d_mask = nc.dram_tensor("mask", [N, G, L, N],
                            f8 if MASK8_DMA else f32r, kind="ExternalInput")
    d_A = nc.dram_tensor("A", [128, L, HC, MSG], f32r, kind="ExternalInput")
    d_wih = nc.dram_tensor("wih", [128, MC, 3 * H], f32r, kind="ExternalInput")
    d_whh8 = nc.dram_tensor("whh8", [128, HC, 3 * H], f8, kind="ExternalInput")
    CR = N + TARGET  # packed const rows: ones|ones12
    d_crows = nc.dram_tensor("crows", [1, CR], f32r, kind="ExternalInput")
    # fp8 GRU bias rows (x16), block 1 all-zero (DoubleRow zero-pair partner)
    d_crows8 = nc.dram_tensor("crows8", [1, 2, 4 * H], f8, kind="ExternalInput")
    d_ones8 = nc.dram_tensor("ones8", [1, 2, N], f8, kind="ExternalInput")
    d_identcol = nc.dram_tensor("identcol", [128, 129], f32r, kind="ExternalInput")
    d_rowb = nc.dram_tensor("rowb", [128, 17, 128], f32r, kind="ExternalInput")
    d_row3 = nc.dram_tensor("row3", [128, 2, TARGET], f32r, kind="ExternalInput")
    d_robias = nc.dram_tensor("robias", [128, 8], f32, kind="ExternalInput")
    d_rob12 = nc.dram_tensor("rob12", [TARGET, 2], f32, kind="ExternalInput")
    d_out = nc.dram_tensor("out", [TARGET, G], f32, kind="ExternalOutput")

    with tile.TileContext(nc) as tc:
        with tc.tile_pool(name="st", bufs=1) as st, \
             tc.tile_pool(name="state", bufs=1) as stt, \
             tc.tile_pool(name="wk", bufs=2) as wk, \
             tc.tile_pool(name="ps", bufs=1, space="PSUM") as ps:

            # ---- static loads, in consumption order ----
            # first PE ops need hT0 + identcol + A[l, hc=0] + mask; then the
            # step-0 GRU needs wih/whh8/bias rows; the rest can trickle in
            boot_t = st.tile([128, N + 2 * MSG], f32r, tag="boot")
            nc.sync.dma_start(boot_t[:], d_boot[:])
            boot_hT0g0 = boot_t[:, 0:N]
            boot_A0 = [boot_t[:, N:N + MSG], boot_t[:, N + MSG:N + 2 * MSG]]
            hT0_sb = st.tile([F_IN, G, N], f32r, tag="hT0")
            A_sb = st.tile([128, L, HC, MSG], f32r, tag="A")
            nc.sync.dma_start(A_sb[:, 2, 0, :], d_A[:, 2, 0, :])
            nc.sync.dma_start(A_sb[:, 3, 0, :], d_A[:, 3, 0, :])
            nc.sync.dma_start(hT0_sb[:], d_hT0[:])
            nc.sync.dma_start(A_sb[:, 0, 0, :], d_A[:, 0, 0, :])
            nc.sync.dma_start(A_sb[:, 1, 0, :], d_A[:, 1, 0, :])
            identcol_t = st.tile([128, 129], f32r, tag="identcol")
            nc.sync.dma_start(identcol_t[:], d_identcol[:])
            ident_sb = identcol_t[:, 0:128]
            onescol_sb = identcol_t[:, 128:129]
            mask_sb = st.tile([N, G, L, N], f32r, tag="mask")
            if MASK8_DMA:
                mask8st = st.tile([N, 2, L, N], f8, tag="mask8st")
                for g_ in range(G):
                    sl8 = g_ % 2
                    nc.sync.dma_start(mask8st[:, sl8, :, :],
                                      d_mask[:, g_, :, :])
                    nc.gpsimd.tensor_scalar_mul(mask_sb[:, g_, :, :],
                                                mask8st[:, sl8, :, :], 1.0)
            else:
                for g_ in range(G):
                    nc.sync.dma_start(mask_sb[:, g_, :, :],
                                      d_mask[:, g_, :, :])
            wih_sb = st.tile([128, MC, 3 * H], f32r, tag="wih")
            for c in range(MC):
                nc.sync.dma_start(wih_sb[:, c, :], d_wih[:, c, :])
            whh8_sb = st.tile([128, HC, 3 * H], f8, tag="whh8")
            nc.sync.dma_start(whh8_sb[:], d_whh8[:])
            crows8_t = st.tile([1, 2, 4 * H], f8, tag="crows8")
            nc.sync.dma_start(crows8_t[:], d_crows8[:])
            ones8_t = st.tile([1, 2, N], f8, tag="ones8")
            nc.sync.dma_start(ones8_t[:], d_ones8[:])
            h_all = stt.tile([N, G, H], f32r, tag="h_all")
            nc.sync.dma_start(h_all[:, :, 0:F_IN], d_h0[:])
            nc.gpsimd.memset(h_all[:, :, F_IN:H].bitcast(f32), 0.0)
            hT_fb = [stt.tile([128, GPB, H], f32r, tag=f"hT_fb{fb_}",
                              name=f"hT_fb{fb_}") for fb_ in range(FB)]
            hT8_all = stt.tile([128, G, HC, 128], f8, tag="hT8_all")
            hT08 = stt.tile([128, G, 2, 128], f8, tag="hT08")
            nc.gpsimd.memset(hT08[:, :, 1, :], 0.0)
            nc.gpsimd.tensor_scalar_mul(hT08[:, :, 0, :], hT0_sb[:], 1.0)
            crows_t = st.tile([1, CR], f32r, tag="crows")
            nc.sync.dma_start(crows_t[:], d_crows[:])
            ones12_sb = crows_t[:, N:N + TARGET]
            for hc_ in range(1, HC):
                for l_ in range(L):
                    nc.sync.dma_start(A_sb[:, l_, hc_, :], d_A[:, l_, hc_, :])

            rowb_t = st.tile([128, 17, 128], f32r, tag="rowb")
            nc.sync.dma_start(rowb_t[:], d_rowb[:])
            r1w0_sb = rowb_t[:, 0:5, :]
            r1w1_sb = rowb_t[:, 5:7, :]
            r1w2_sb = rowb_t[:, 7:9, :]
            r2w0_sb = rowb_t[:, 9:13, :]
            r2w1_sb = rowb_t[:, 13:15, :]
            r2w2_sb = rowb_t[:, 15:17, :]
            row3_t = st.tile([128, 2, TARGET], f32r, tag="row3")
            nc.sync.dma_start(row3_t[:], d_row3[:])
            r1w3_sb = row3_t[:, 0, :]
            r2w3_sb = row3_t[:, 1, :]
            robias_t = st.tile([128, 8], f32, tag="robias")
            nc.sync.dma_start(robias_t[:], d_robias[:])
            r1b0_sb = robias_t[:, 0:1]
            r1b1_sb = robias_t[:, 1:3]
            r1b2_sb = robias_t[:, 3:4]
            r2b0_sb = robias_t[:, 4:5]
            r2b1_sb = robias_t[:, 5:7]
            r2b2_sb = robias_t[:, 7:8]
            rob12_t = st.tile([TARGET, 2], f32, tag="rob12")
            nc.sync.dma_start(rob12_t[:], d_rob12[:])
            r1b3_sb = rob12_t[:, 0:1]
            r2b3_sb = rob12_t[:, 1:2]


            for _rep in range(nreps):
                # NOTE: the reference's per-step node_mask multiply is
                # dropped: masked inputs guarantee no edges touch virtual
                # nodes, their per-node GRU lanes never mix into real nodes,
                # and the readout re-applies mask_row.

                # ---- message passing loop ----
                def hT_chunk(s, g, hc):
                    if s == 0:
                        assert hc == 0
                        return hT0_sb[:, g, :]
                    return hT_fb[g // GPB][:, g % GPB,
                                           hc * 128:(hc + 1) * 128]

                pending_htr = []  # delayed h-transposes (PE in-order relief)

                def h_transpose(s, g):
                    # h^T for next step / readout; one graph behind the GRU
                    # so the PE never head-of-line blocks on g's update chain
                    tp2 = ps.tile([128, H], f32r, tag="pMT", bufs=2,
                                  name=f"tp2_{s}_{g}")
                    for c in range(HC):
                        nc.tensor.transpose(tp2[:, c * 128:(c + 1) * 128],
                                            h_all[:, g, c * 128:(c + 1) * 128],
                                            ident_sb[:])
                    hTg = hT_fb[g // GPB][:, g % GPB, :]
                    if g % 2 == 0:
                        nc.vector.tensor_copy(hTg, tp2[:])
                    else:
                        nc.scalar.copy(hTg, tp2[:])
                    if s < NSTEP - 1:  # fp8 h^T for next step's gh GEMM
                        # (gpsimd cannot read PSUM -> source the SBUF copy)
                        nc.gpsimd.tensor_scalar_mul(hT8_all[:, g, :, :],
                                                    hTg, 1.0)

                # ---- readout (layer-major over 4 independent chains) ----
                out_sb = st.tile([TARGET, G], f32, tag="out_sb")
                nfree = GPB * N  # 512
                r1_ws = [[r1w0_sb[:, kc, :] for kc in range(5)],
                         [r1w1_sb[:, oc, :] for oc in range(2)],
                         [r1w2_sb[:, kc, :] for kc in range(2)],
                         r1w3_sb[:]]
                r1_bs = [r1b0_sb[:],
                         [r1b1_sb[:, oc:oc + 1] for oc in range(2)],
                         r1b2_sb[:]]
                r2_ws = [[r2w0_sb[:, kc, :] for kc in range(4)],
                         [r2w1_sb[:, oc, :] for oc in range(2)],
                         [r2w2_sb[:, kc, :] for kc in range(2)],
                         r2w3_sb[:]]
                r2_bs = [r2b0_sb[:],
                         [r2b1_sb[:, oc:oc + 1] for oc in range(2)],
                         r2b2_sb[:]]
                chains = []
                for fb in range(FB):
                    gsl = slice(fb * GPB, (fb + 1) * GPB)
                    h_in_chunks = [hT_fb[fb][:, :, kc * 128:(kc + 1) * 128]
                                   for kc in range(HC)]
                    chains.append(dict(fb=fb, w="g", ws=r1_ws, bs=r1_bs,
                                       ins=h_in_chunks + [hT0_sb[:, gsl, :]]))
                    chains.append(dict(fb=fb, w="v", ws=r2_ws, bs=r2_bs,
                                       ins=h_in_chunks))
                relueng = [None, nc.vector, None, nc.vector]

                def relu_from(dst, src, bias, ci):
                    eng = relueng[ci]
                    if eng is None:
                        nc.scalar.activation(dst, src, AF.Relu, bias=bias)
                    else:
                        eng.tensor_scalar(dst, src, bias, 0.0,
                                          op0=ALU.add, op1=ALU.max)

                def ro_l0(ch, ci):
                    key = f"{ch['w']}{ch['fb']}"
                    p = ps.tile([128, nfree], f32,
                                tag="pP" if ci % 2 == 0 else "pG2", bufs=2,
                                name=f"rop0_{key}")
                    for i, (wap, rhs) in enumerate(zip(ch["ws"][0], ch["ins"])):
                        nc.tensor.matmul(p[:], wap, rhs, start=(i == 0),
                                         stop=(i == len(ch["ins"]) - 1))
                    a1 = wk.tile([128, nfree], f32r, tag="P", bufs=8,
                                 name=f"roa1_{key}")
                    relu_from(a1[:], p[:], ch["bs"][0], ci)
                    ch["a1"] = a1

                # fb=0 L0 first: it only needs graphs 0-3, so the PE can run
                # it while graph 7's update chain is still in flight; the
                # last h-transpose is flushed in between
                def ro_l1(fb):
                    for ci0, ch in enumerate(chains[2 * fb:2 * fb + 2]):
                        ci = 2 * fb + ci0
                        key = f"{ch['w']}{ch['fb']}"
                        ch["a2"] = []
                        for oc in range(2):
                            p2 = ps.tile([128, nfree], f32,
                                         tag="pP" if oc == 0 else "pG2",
                                         bufs=2, name=f"rop1_{key}_{oc}")
                            nc.tensor.matmul(p2[:], ch["ws"][1][oc],
                                             ch["a1"][:],
                                             start=True, stop=True)
                            t = wk.tile([128, nfree], f32r, tag="P", bufs=8,
                                        name=f"roa2_{key}_{oc}")
                            relu_from(t[:], p2[:], ch["bs"][1][oc],
                                      (ci + oc) % 2)
                            ch["a2"].append(t)

                def ro_l2(fb):
                    for ci0, ch in enumerate(chains[2 * fb:2 * fb + 2]):
                        ci = 2 * fb + ci0
                        key = f"{ch['w']}{ch['fb']}"
                        p3 = ps.tile([128, nfree], f32,
                                      tag="pP" if ci0 == 0 else "pG2",
                                      bufs=2, name=f"rop2_{key}")
                        for kc in range(2):
                            nc.tensor.matmul(p3[:], ch["ws"][2][kc],
                                             ch["a2"][kc][:],
                                             start=(kc == 0), stop=(kc == 1))
                        a3 = wk.tile([128, nfree], f32r, tag="P", bufs=8,
                                     name=f"roa3_{key}")
                        # always Act: the DVE queue backs up at the tail
                        relu_from(a3[:], p3[:], ch["bs"][2], 0)
                        ch["a3"] = a3

                def ro_l3(fb):
                    # v-chain first: its PSUM feeds the DVE vm op while the
                    # g-chain's L3 + sigmoid still run
                    for ch in chains[2 * fb:2 * fb + 2][::-1]:
                        key = f"{ch['w']}{ch['fb']}"
                        p4 = ps.tile([TARGET, nfree], f32, tag="pGN", bufs=2,
                                     name=f"rop3_{key}")
                        nc.tensor.matmul(p4[:], ch["ws"][3], ch["a3"][:],
                                         start=True, stop=True)
                        ch["p4"] = p4

                def ro_finals(fb):
                    # finals: sum_v gate*val*mask per graph
                    fsl = slice(fb * nfree, (fb + 1) * nfree)
                    chg, chv = chains[2 * fb:2 * fb + 2]
                    # val*mask fused: (p4 + b) * mb, straight from PSUM
                    vm = wk.tile([TARGET, nfree], f32, tag="z", bufs=2,
                                 name=f"vm_{fb}")
                    nc.vector.scalar_tensor_tensor(
                        vm[:], chv["p4"][:], r2b3_sb[:], mb_sb[:, fsl],
                        op0=ALU.add, op1=ALU.mult)
                    gate_s = wk.tile([TARGET, nfree], f32, tag="r", bufs=2,
                                     name=f"gate_{fb}")
                    nc.scalar.activation(gate_s[:], chg["p4"][:], AF.Sigmoid,
                                         bias=r1b3_sb[:])
                    pr2 = wk.tile([TARGET, GPB, N], f32, tag="t1", bufs=2,
                                  name=f"pr2_{fb}")
                    nc.vector.scalar_tensor_tensor(
                        pr2[:], gate_s[:], 1.0, vm[:],
                        op0=ALU.mult, op1=ALU.mult)
                    nc.vector.tensor_reduce(
                        out_sb[:, fb * GPB:(fb + 1) * GPB], pr2[:],
                        axis=AX.X, op=ALU.add)

                for s in range(NSTEP):
                    hcs = [0] if s == 0 else list(range(HC))
                    # -- phase 1 (all graphs): projections + aggregation -> mT --
                    def projections(g):
                        P_sb = []
                        cpeng = [nc.vector.tensor_copy, nc.scalar.copy,
                                 nc.scalar.copy, nc.scalar.copy]
                        if s == 0:
                            # step-0 phase 1 is copy-bound on Act: rebalance
                            # one PSUM->SBUF copy onto DVE
                            cpeng[2] = nc.vector.tensor_copy
                        for l in range(L):
                            # step 0: spread projection PSUM across the pP
                            # and (GRU-idle) pG2 pools -> 4 effective bufs,
                            # so short s0 projections don't wait on copies
                            ptag = "pP" if l % 2 == 0 else "pG2"
                            pp = ps.tile([128, MSG], f32, tag=ptag, bufs=2,
                                         name=f"pp_{s}_{g}_{l}")
                            if s == 0 and g == 0 and l < 2:
                                # sourced from the packed boot DMA: the very
                                # first matmuls wait on one small transfer
                                nc.tensor.matmul(pp[:], boot_hT0g0,
                                                 boot_A0[l],
                                                 start=True, stop=True)
                                psb = wk.tile([128, MSG], f32r, tag="P",
                                              bufs=8, name=f"psb_{s}_{g}_{l}")
                                cpeng[l](psb[:], pp[:])
                                P_sb.append(psb)
                                continue
                            for i, hc in enumerate(hcs):
                                nc.tensor.matmul(pp[:], hT_chunk(s, g, hc),
                                                 A_sb[:, l, hc, :],
                                                 start=(i == 0),
                                                 stop=(i == len(hcs) - 1))
                            psb = wk.tile([128, MSG], f32r, tag="P", bufs=8,
                                          name=f"psb_{s}_{g}_{l}")
                            cpeng[l](psb[:], pp[:])
                            P_sb.append(psb)
                        return P_sb

                    def agg_m(g, P_sb):
                        mp = ps.tile([128, MSG], f32, tag="pMT", bufs=2,
                                     name=f"mp_{s}_{g}")
                        for l in range(L):
                            nc.tensor.matmul(mp[:], mask_sb[:, g, l, :], P_sb[l][:],
                                             start=(l == 0), stop=(l == L - 1))
                        m_sb = wk.tile([128, MSG], f32r, tag="m", bufs=2,
                                       name=f"m_{s}_{g}")
                        nc.vector.tensor_copy(m_sb[:], mp[:])
                        return m_sb

                    def mT_from(g, m_sb):
                        tp = ps.tile([128, MSG], f32r, tag="pMT", bufs=2,
                                     name=f"tp_{s}_{g}")
                        for c in range(MC):
                            nc.tensor.transpose(tp[:, c * 128:(c + 1) * 128],
                                                m_sb[:, c * 128:(c + 1) * 128],
                                                ident_sb[:])
                        mT_sb = wk.tile([128, MSG], f32r, tag="mT", bufs=6,
                                        name=f"mT_{s}_{g}")
                        nc.scalar.copy(mT_sb[:], tp[:])
                        return mT_sb

                    # last step runs fb1's graphs first so their h^T land
                    # early for the (fb1-first) readout
                    gorder = (list(range(G)) if s < NSTEP - 1
                              else list(range(GPB, G)) + list(range(GPB)))
                    mT_tiles = {}
                    for gp in range(G // 2):
                        g0, g1 = gorder[2 * gp], gorder[2 * gp + 1]
                        Ps0 = projections(g0)
                        if gp == 0 and pending_htr:
                            h_transpose(*pending_htr.pop())
                        Ps1 = projections(g1)
                        mT_tiles[g0] = mT_from(g0, agg_m(g0, Ps0))
                        mT_tiles[g1] = mT_from(g1, agg_m(g1, Ps1))
                    if s == 0:
                        # read mask + its 12-row broadcast; emitted here to
                        # keep it off the startup critical path (only the
                        # readout finals consume these)
                        mask_row = st.tile([1, G * N], f32r, tag="mask_row")
                        mb_sb = st.tile([TARGET, G * N], f32, tag="mb_sb")
                        for fb in range(FB):
                            fsl = slice(fb * GPB * N, (fb + 1) * GPB * N)
                            colsum = ps.tile([1, GPB * N], f32, tag="pMT",
                                             bufs=2, name=f"colsum{fb}")
                            nc.tensor.matmul(
                                colsum[:], onescol_sb[:],
                                hT0_sb[:, fb * GPB:(fb + 1) * GPB, :],
                                start=True, stop=True)
                            nc.vector.tensor_scalar(mask_row[:, fsl],
                                                    colsum[:], 0.0, None,
                                                    op0=ALU.not_equal)
                            mbp = ps.tile([TARGET, GPB * N], f32, tag="pMT",
                                          bufs=2, name=f"mbp{fb}")
                            nc.tensor.matmul(mbp[:], ones12_sb[:],
                                             mask_row[:, fsl],
                                             start=True, stop=True)
                            nc.scalar.copy(mb_sb[:, fsl], mbp[:])
                    # -- phase 2 (all graphs): GRU GEMMs + gates --
                    def gh8_pairs(g, sl):
                        # fp8 DoubleRow h @ Whh chunk-pairs (zero-pair at s=0)
                        if s == 0:
                            return [(hT08[:, g, :, :], whh8_sb[:, 0:2, sl])]
                        return [(hT8_all[:, g, c:c + 2, :],
                                 whh8_sb[:, c:c + 2, sl]) for c in (0, 2)]

                    def gate_mms(o, dr_mms, f32_mms):
                        # fp8-DR insts first: they depend only on hT8/bias
                        # rows, so the PE can start them before the mT copy
                        # for this graph has landed
                        ntot = len(f32_mms) + len(dr_mms)
                        i = 0
                        for lh, rh in dr_mms:
                            nc.tensor.matmul(o, lh, rh, start=(i == 0),
                                             stop=(i == ntot - 1),
                                             perf_mode=DR)
                            i += 1
                        for lh, rh in f32_mms:
                            nc.tensor.matmul(o, lh, rh, start=(i == 0),
                                             stop=(i == ntot - 1))
                            i += 1

                    for gi_, g in enumerate(gorder):
                        mT_sb = mT_tiles[g]
                        # 4) GRU gate GEMMs: gi/mT in f32r, gh + biases in
                        # fp8 DoubleRow; PSUM carries GSC x the true preacts
                        rzp = [ps.tile([128, H], f32, tag="pG2", bufs=2,
                                       name=f"g{'rz'[h_]}_{s}_{g}")
                               for h_ in range(2)]
                        gin = ps.tile([128, H], f32, tag="pGN", bufs=2,
                                      name=f"gin_{s}_{g}")
                        ghn = ps.tile([128, H], f32, tag="pGN", bufs=2,
                                      name=f"ghn_{s}_{g}")
                        gate_mms(
                            ghn[:],
                            gh8_pairs(g, slice(2 * H, 3 * H))
                            + [(ones8_t[:], crows8_t[:, :, 3 * H:4 * H])],
                            [])
                        for half in range(2):
                            sl = slice(half * H, (half + 1) * H)
                            gate_mms(
                                rzp[half][:],
                                gh8_pairs(g, sl)
                                + [(ones8_t[:], crows8_t[:, :, sl])],
                                [(mT_sb[:, c * 128:(c + 1) * 128],
                                  wih_sb[:, c, sl]) for c in range(MC)])
                        gate_mms(
                            gin[:],
                            [(ones8_t[:], crows8_t[:, :, 2 * H:3 * H])],
                            [(mT_sb[:, c * 128:(c + 1) * 128],
                              wih_sb[:, c, 2 * H:3 * H]) for c in range(MC)])
                        # 5) gate nonlinearities + state update as
                        # h' = (1-z)*n + z*h: z*h runs before tanh lands,
                        # so only 2 DVE ops trail the tanh
                        r_sb = wk.tile([128, H], f32, tag="r", bufs=2,
                                       name=f"r_{s}_{g}")
                        nc.scalar.activation(r_sb[:], rzp[0][:], AF.Sigmoid,
                                             scale=1.0 / GSC)
                        z_sb = wk.tile([128, H], f32, tag="z", bufs=2,
                                       name=f"z_{s}_{g}")
                        nc.scalar.activation(z_sb[:], rzp[1][:],
                                             AF.Sigmoid, scale=1.0 / GSC)
                        # u = 1-z via negated-scale sigmoid
                        u_sb = wk.tile([128, H], f32, tag="u", bufs=2,
                                       name=f"u_{s}_{g}")
                        nc.scalar.activation(u_sb[:], rzp[1][:], AF.Sigmoid,
                                             scale=-1.0 / GSC)
                        rhn = wk.tile([128, H], f32, tag="t1", bufs=2,
                                      name=f"rhn_{s}_{g}")
                        nc.vector.tensor_mul(rhn[:], r_sb[:], ghn[:])
                        zh = wk.tile([128, H], f32, tag="zh", bufs=2,
                                     name=f"zh_{s}_{g}")
                        nc.vector.tensor_mul(zh[:], z_sb[:], h_all[:, g, :])
                        npre = wk.tile([128, H], f32, tag="t2", bufs=2,
                                       name=f"npre_{s}_{g}")
                        nc.vector.tensor_add(npre[:], rhn[:], gin[:])
                        n_sb = wk.tile([128, H], f32, tag="n", bufs=2,
                                       name=f"n_{s}_{g}")
                        nc.scalar.activation(n_sb[:], npre[:], AF.Tanh,
                                             scale=1.0 / GSC)
                        un = wk.tile([128, H], f32, tag="t1", bufs=2,
                                     name=f"un_{s}_{g}")
                        nc.vector.tensor_mul(un[:], u_sb[:], n_sb[:])
                        nc.vector.tensor_add(h_all[:, g, :], un[:], zh[:])
                        pending_htr.append((s, g))
                        if s == NSTEP - 1 and gi_ == G - 1:
                            # fb1's h^T is complete (graphs 4-7 ran first +
                            # their transposes flushed): overlap its readout
                            # L0..L2 with the remaining fb0 GRU work
                            ro_l0(chains[2], 2)
                            ro_l0(chains[3], 3)
                            ro_l1(1)
                            ro_l2(1)
                        if gi_ >= 1:
                            h_transpose(*pending_htr.pop(0))

                # zippered, fb1 first (its graphs ran first in step 3):
                # fb1's L0 covers the last fb0 graph's update chain, then
                # layers alternate fbs so each fb's relu latency hides
                # under the other fb's GEMMs
                while pending_htr:
                    h_transpose(*pending_htr.pop(0))
                ro_l0(chains[0], 0)
                ro_l0(chains[1], 1)
                ro_l3(1)
                ro_l1(0)
                ro_finals(1)
                ro_l2(0)
                ro_l3(0)
                ro_finals(0)
                nc.sync.dma_start(d_out[:], out_sb[:])


    nc.compile()
    return nc


def _prep_core_inputs(core, g_, h_in, e, A, gru_Wih, gru_Whh, gru_bih,
                      gru_bhh, r1_Ws, r1_bs, r2_Ws, r2_bs):
    cs = slice(core * G, (core + 1) * G)
    f = np.float32
    h0 = np.asarray(h_in[cs], f)
    hT0 = np.ascontiguousarray(h_in[cs].transpose(2, 0, 1))  # [F, G, N]
    labels = np.arange(1, L + 1, dtype=f)
    # mask[w, g, l, v] = (e[g, v, w] == l+1)
    e_c = e[cs]  # [G, V, W]
    oh = (e_c[:, None, :, :] == labels[None, :, None, None]).astype(f)  # [G,L,V,W]
    mask = np.ascontiguousarray(oh.transpose(3, 0, 1, 2))  # [W, G, L, V]
    if MASK8_DMA:
        import ml_dtypes
        mask = mask.astype(ml_dtypes.float8_e4m3)
    boot = np.concatenate([hT0[:, 0, :], np.asarray(A[0, 0:128, :], f),
                           np.asarray(A[1, 0:128, :], f)], 1)
    return {
        "h0": np.ascontiguousarray(h0.transpose(1, 0, 2)),  # [N, G, F_IN]
        "hT0": hT0,
        "mask": mask,
        "boot": np.ascontiguousarray(boot),
    }


def _prep_shared_inputs(A, gru_Wih, gru_Whh, gru_bih, gru_bhh,
                        r1_Ws, r1_bs, r2_Ws, r2_bs):
    f = np.float32

    def chunk_rows(M, nch):  # [K, C] -> [128, nch, C] with K = nch*128
        K, C = M.shape
        assert K == nch * 128
        return np.ascontiguousarray(M.reshape(nch, 128, C).transpose(1, 0, 2))

    import ml_dtypes
    f8np = ml_dtypes.float8_e4m3
    GSC = np.float32(16.0)

    A_t = np.ascontiguousarray(
        A.reshape(L, HC, 128, MSG).transpose(2, 0, 1, 3))  # [128, L, HC, MSG]
    wih = chunk_rows(np.ascontiguousarray(gru_Wih.T) * GSC, MC)  # [128,MC,3H]
    whh8 = chunk_rows(np.ascontiguousarray(gru_Whh.T) * GSC, HC).astype(f8np)
    brz = (gru_bih + gru_bhh)[:2 * H].astype(f)
    bin_ = gru_bih[2 * H:].astype(f)
    bhn = gru_bhh[2 * H:].astype(f)
    crows8 = np.zeros((1, 2, 4 * H), f)
    crows8[0, 0, :] = np.concatenate([brz, bin_, bhn]) * GSC
    ones8 = np.ones((1, 2, N), f8np)

    # readout weights, transposed layout
    r1w0t = np.ascontiguousarray(r1_Ws[0].T)  # [2H, 128]
    r1w0 = np.zeros((128, 5, 128), f)
    for kc in range(4):
        r1w0[:, kc, :] = r1w0t[kc * 128:(kc + 1) * 128]
    r1w0[:, 4, :] = r1w0t[H:H + F_IN]  # h0 chunk (features 0:128 of h0 half)
    r1w1 = np.ascontiguousarray(r1_Ws[1].T.reshape(128, 2, 128))
    r1w2 = chunk_rows(np.ascontiguousarray(r1_Ws[2].T), 2)
    r1w3 = np.ascontiguousarray(r1_Ws[3].T)  # [128, 12]
    r2w0 = chunk_rows(np.ascontiguousarray(r2_Ws[0].T), 4)
    r2w1 = np.ascontiguousarray(r2_Ws[1].T.reshape(128, 2, 128))
    r2w2 = chunk_rows(np.ascontiguousarray(r2_Ws[2].T), 2)
    r2w3 = np.ascontiguousarray(r2_Ws[3].T)

    crows = np.concatenate([np.ones(N, f), np.ones(TARGET, f)])[None, :]
    identcol = np.concatenate([np.eye(128, dtype=f), np.ones((128, 1), f)], 1)
    rowb = np.concatenate([r1w0, r1w1, r1w2, r2w0, r2w1, r2w2], axis=1)
    row3 = np.stack([r1w3, r2w3], axis=1)
    robias = np.concatenate([
        r1_bs[0].reshape(-1, 1).astype(f),
        np.ascontiguousarray(r1_bs[1].reshape(2, 128).T),
        r1_bs[2].reshape(-1, 1).astype(f),
        r2_bs[0].reshape(-1, 1).astype(f),
        np.ascontiguousarray(r2_bs[1].reshape(2, 128).T),
        r2_bs[2].reshape(-1, 1).astype(f)], axis=1)
    rob12 = np.concatenate([r1_bs[3].reshape(-1, 1).astype(f),
                            r2_bs[3].reshape(-1, 1).astype(f)], axis=1)
    return {
        "A": A_t, "wih": wih, "whh8": np.ascontiguousarray(whh8),
        "crows": np.ascontiguousarray(crows),
        "crows8": crows8.astype(f8np),
        "ones8": ones8,
        "identcol": np.ascontiguousarray(identcol),
        "rowb": np.ascontiguousarray(rowb),
        "row3": np.ascontiguousarray(row3),
        "robias": np.ascontiguousarray(robias),
        "rob12": np.ascontiguousarray(rob12),
    }


def _get_nc(nreps=1):
    key = ("nc", nreps)
    if key not in _CACHE:
        _CACHE[key] = _build(nreps)
    return _CACHE[key]


def _run(in_maps, **kwargs):
    nc = _get_nc()
    return run_bass_kernel_spmd(nc, in_maps, core_ids=list(range(NCORES)),
                                **kwargs)


def make_in_maps(g, h_in, e, A, gru_Wih, gru_Whh, gru_bih, gru_bhh,
                 r1_W0, r1_b0, r1_W1, r1_b1, r1_W2, r1_b2, r1_W3, r1_b3,
                 r2_W0, r2_b0, r2_W1, r2_b1, r2_W2, r2_b2, r2_W3, r2_b3):
    r1_Ws, r1_bs = [r1_W0, r1_W1, r1_W2, r1_W3], [r1_b0, r1_b1, r1_b2, r1_b3]
    r2_Ws, r2_bs = [r2_W0, r2_W1, r2_W2, r2_W3], [r2_b0, r2_b1, r2_b2, r2_b3]
    arrs = {k: np.asarray(v, np.float32) for k, v in dict(
        g=g, h_in=h_in, e=e, A=A, gru_Wih=gru_Wih, gru_Whh=gru_Whh,
        gru_bih=gru_bih, gru_bhh=gru_bhh).items()}
    r1_Ws = [np.asarray(w, np.float32) for w in r1_Ws]
    r1_bs = [np.asarray(b, np.float32) for b in r1_bs]
    r2_Ws = [np.asarray(w, np.float32) for w in r2_Ws]
    r2_bs = [np.asarray(b, np.float32) for b in r2_bs]
    shared = _prep_shared_inputs(arrs["A"], arrs["gru_Wih"], arrs["gru_Whh"],
                                 arrs["gru_bih"], arrs["gru_bhh"],
                                 r1_Ws, r1_bs, r2_Ws, r2_bs)
    in_maps = []
    for core in range(NCORES):
        m = dict(shared)
        m.update(_prep_core_inputs(core, arrs["g"], arrs["h_in"], arrs["e"],
                                   arrs["A"], arrs["gru_Wih"], arrs["gru_Whh"],
                                   arrs["gru_bih"], arrs["gru_bhh"],
                                   r1_Ws, r1_bs, r2_Ws, r2_bs))
        in_maps.append(m)
    return in_maps


def kernel(**inputs):
    in_maps = make_in_maps(**inputs)
    res = _run(in_maps)
    out = np.zeros((B, TARGET), np.float32)
    for core in range(NCORES):
        out[core * G:(core + 1) * G] = res.results[core]["out"].T
    return out


if __name__ == "__main__":
    import reference
    inputs = {k: np.asarray(v) for k, v in reference.setup_inputs().items()}
    expected = np.asarray(reference.reference(**inputs))
    actual = kernel(**inputs)
    scale = np.abs(expected).max()
    err = np.abs(actual - expected).max() / scale
    print("Relative error:", err)



# revision 130
# speedup vs baseline: 1.0013x; 1.0012x over previous
"""MPNN-GGNN forward on 8 Trainium2 NeuronCores.

Data-parallel over the batch: 8 graphs per core. All weights replicated.
Per-core Bass/Tile kernel computes 4 message-passing + GRU steps and the
gated readout entirely on-chip; f32r (full fp32 bits, reduced-precision
multiplier) matmuls at full PE rate. The small-magnitude gh GEMM
(h @ Whh^T) and GRU bias rows run as fp8e4 DoubleRow matmuls (2 K-chunks
per instruction at 0.5 cycles/row = 4x f32r); the error-sensitive
message path (proj/agg/gi, preact sigma ~50) must stay f32r (measured:
fp8 there = 0.07 rel err, bf16 = 0.026, vs the 2e-2 gate).

Schedule notes (PE is in-order; head-of-line blocking is the enemy):
  - h-transposes run one graph behind their update chain
  - fp8-DR GRU matmuls are emitted before the mT-dependent f32r ones
  - last step runs fb1's graphs first; fb1's readout L0..L2 is injected
    into the final GRU iteration, and readout layers zipper between fbs
  - per-step node_mask multiply dropped (masked inputs make it redundant
    until the readout's mask_row)

Layout conventions per core (G = 8 graphs, N = 128 nodes, H = MSG = 512):
  h_all  [128(node), G, 512]  fp32   node-major hidden state
  hT_fb[fb] [128(feat), 4, 512]  h^T split per readout block so fb0's
         readout only depends on graphs 0-3
  hT8_all[128, G, HC, 128] fp8   h^T chunk-pairs for DoubleRow gh
  mask_sb[128(w),   G, L, 128(v)]    (e^T == l+1) one-hot adjacency
  matmul convention: out[i,j] = sum_k lhsT[k,i] * rhs[k,j]
"""

import numpy as np

import concourse.mybir as mybir
import concourse.tile as tile
from concourse import bacc
from concourse.bass_utils import run_bass_kernel_spmd

# problem constants (hardcoded per contract)
B, N, F_IN = 64, 128, 128
H, MSG, L = 512, 512, 4
NSTEP = 4
TARGET = 12
NCORES = 8
G = B // NCORES          # graphs per core
HC = H // 128            # h chunks
MC = MSG // 128          # msg chunks
FB = 2                   # readout free blocks (4 graphs x 128 nodes each)
GPB = G // FB

f32 = mybir.dt.float32
f32r = mybir.dt.float32r
f8 = mybir.dt.float8e4
AF = mybir.ActivationFunctionType
ALU = mybir.AluOpType
AX = mybir.AxisListType
DR = mybir.MatmulPerfMode.DoubleRow
GSC = 16.0  # gate-preact PSUM scale: wih/whh/biases premultiplied by 16
# Direct-mT aggregation is dead: hw forbids mixing f32r with 8/16-bit
# matmul inputs, f32r x f32r at free=128 pays the 4x narrow-moving-dim
# penalty, and a bf16 P loses too much precision (rel err 0.026 measured).
# The 0/1 mask still ships as fp8 (4x less serial prologue DMA) and is
# expanded to f32r on the idle Pool engine.
MASK8_DMA = True

_CACHE = {}


def _build(nreps=1):
    nc = bacc.Bacc("TRN2", target_bir_lowering=False)

    # ---- DRAM I/O ----
    # boot: hT0 for graph 0 + A[l=0..1, hc=0] packed in one early DMA
    d_boot = nc.dram_tensor("boot", [128, N + 2 * MSG], f32r,
                            kind="ExternalInput")
    d_h0 = nc.dram_tensor("h0", [N, G, F_IN], f32r, kind="ExternalInput")
    d_hT0 = nc.dram_tensor("hT0", [F_IN, G, N], f32r, kind="ExternalInput")
    d_mask = nc.dram_tensor("mask", [N, G, L, N],
                            f8 if MASK8_DMA else f32r, kind="ExternalInput")
    d_A = nc.dram_tensor("A", [128, L, HC, MSG], f32r, kind="ExternalInput")
    d_wih = nc.dram_tensor("wih", [128, MC, 3 * H], f32r, kind="ExternalInput")
    d_whh8 = nc.dram_tensor("whh8", [128, HC, 3 * H], f8, kind="ExternalInput")
    CR = N + TARGET  # packed const rows: ones|ones12
    d_crows = nc.dram_tensor("crows", [1, CR], f32r, kind="ExternalInput")
    # fp8 GRU bias rows (x16), block 1 all-zero (DoubleRow zero-pair partner)
    d_crows8 = nc.dram_tensor("crows8", [1, 2, 4 * H], f8, kind="ExternalInput")
    d_ones8 = nc.dram_tensor("ones8", [1, 2, N], f8, kind="ExternalInput")
    d_identcol = nc.dram_tensor("identcol", [128, 129], f32r, kind="ExternalInput")
    d_rowb = nc.dram_tensor("rowb", [128, 17, 128], f32r, kind="ExternalInput")
    d_row3 = nc.dram_tensor("row3", [128, 2, TARGET], f32r, kind="ExternalInput")
    d_robias = nc.dram_tensor("robias", [128, 8], f32, kind="ExternalInput")
    d_rob12 = nc.dram_tensor("rob12", [TARGET, 2], f32, kind="ExternalInput")
    d_out = nc.dram_tensor("out", [TARGET, G], f32, kind="ExternalOutput")

    with tile.TileContext(nc) as tc:
        with tc.tile_pool(name="st", bufs=1) as st, \
             tc.tile_pool(name="state", bufs=1) as stt, \
             tc.tile_pool(name="wk", bufs=2) as wk, \
             tc.tile_pool(name="ps", bufs=1, space="PSUM") as ps:

            # ---- static loads, in consumption order ----
            # first PE ops need hT0 + identcol + A[l, hc=0] + mask; then the
            # step-0 GRU needs wih/whh8/bias rows; the rest can trickle in
            boot_t = st.tile([128, N + 2 * MSG], f32r, tag="boot")
            nc.sync.dma_start(boot_t[:], d_boot[:])
            boot_hT0g0 = boot_t[:, 0:N]
            boot_A0 = [boot_t[:, N:N + MSG], boot_t[:, N + MSG:N + 2 * MSG]]
            hT0_sb = st.tile([F_IN, G, N], f32r, tag="hT0")
            A_sb = st.tile([128, L, HC, MSG], f32r, tag="A")
            nc.sync.dma_start(A_sb[:, 2, 0, :], d_A[:, 2, 0, :])
            nc.sync.dma_start(A_sb[:, 3, 0, :], d_A[:, 3, 0, :])
            nc.sync.dma_start(hT0_sb[:], d_hT0[:])
            nc.sync.dma_start(A_sb[:, 0, 0, :], d_A[:, 0, 0, :])
            nc.sync.dma_start(A_sb[:, 1, 0, :], d_A[:, 1, 0, :])
            identcol_t = st.tile([128, 129], f32r, tag="identcol")
            nc.sync.dma_start(identcol_t[:], d_identcol[:])
            ident_sb = identcol_t[:, 0:128]
            onescol_sb = identcol_t[:, 128:129]
            mask_sb = st.tile([N, G, L, N], f32r, tag="mask")
            if MASK8_DMA:
                mask8st = st.tile([N, 2, L, N], f8, tag="mask8st")
                for g_ in range(G):
                    sl8 = g_ % 2
                    nc.sync.dma_start(mask8st[:, sl8, :, :],
                                      d_mask[:, g_, :, :])
                    nc.gpsimd.tensor_scalar_mul(mask_sb[:, g_, :, :],
                                                mask8st[:, sl8, :, :], 1.0)
            else:
                for g_ in range(G):
                    nc.sync.dma_start(mask_sb[:, g_, :, :],
                                      d_mask[:, g_, :, :])
            wih_sb = st.tile([128, MC, 3 * H], f32r, tag="wih")
            for c in range(MC):
                nc.sync.dma_start(wih_sb[:, c, :], d_wih[:, c, :])
            whh8_sb = st.tile([128, HC, 3 * H], f8, tag="whh8")
            nc.sync.dma_start(whh8_sb[:], d_whh8[:])
            crows8_t = st.tile([1, 2, 4 * H], f8, tag="crows8")
            nc.sync.dma_start(crows8_t[:], d_crows8[:])
            ones8_t = st.tile([1, 2, N], f8, tag="ones8")
            nc.sync.dma_start(ones8_t[:], d_ones8[:])
            h_all = stt.tile([N, G, H], f32r, tag="h_all")
            nc.sync.dma_start(h_all[:, :, 0:F_IN], d_h0[:])
            nc.gpsimd.memset(h_all[:, :, F_IN:H].bitcast(f32), 0.0)
            hT_fb = [stt.tile([128, GPB, H], f32r, tag=f"hT_fb{fb_}",
                              name=f"hT_fb{fb_}") for fb_ in range(FB)]
            hT8_all = stt.tile([128, G, HC, 128], f8, tag="hT8_all")
            hT08 = stt.tile([128, G, 2, 128], f8, tag="hT08")
            nc.gpsimd.memset(hT08[:, :, 1, :], 0.0)
            nc.gpsimd.tensor_scalar_mul(hT08[:, :, 0, :], hT0_sb[:], 1.0)
            crows_t = st.tile([1, CR], f32r, tag="crows")
            nc.sync.dma_start(crows_t[:], d_crows[:])
            ones12_sb = crows_t[:, N:N + TARGET]
            for hc_ in range(1, HC):
                for l_ in range(L):
                    nc.sync.dma_start(A_sb[:, l_, hc_, :], d_A[:, l_, hc_, :])

            rowb_t = st.tile([128, 17, 128], f32r, tag="rowb")
            nc.sync.dma_start(rowb_t[:], d_rowb[:])
            r1w0_sb = rowb_t[:, 0:5, :]
            r1w1_sb = rowb_t[:, 5:7, :]
            r1w2_sb = rowb_t[:, 7:9, :]
            r2w0_sb = rowb_t[:, 9:13, :]
            r2w1_sb = rowb_t[:, 13:15, :]
            r2w2_sb = rowb_t[:, 15:17, :]
            row3_t = st.tile([128, 2, TARGET], f32r, tag="row3")
            nc.sync.dma_start(row3_t[:], d_row3[:])
            r1w3_sb = row3_t[:, 0, :]
            r2w3_sb = row3_t[:, 1, :]
            robias_t = st.tile([128, 8], f32, tag="robias")
            nc.sync.dma_start(robias_t[:], d_robias[:])
            r1b0_sb = robias_t[:, 0:1]
            r1b1_sb = robias_t[:, 1:3]
            r1b2_sb = robias_t[:, 3:4]
            r2b0_sb = robias_t[:, 4:5]
            r2b1_sb = robias_t[:, 5:7]
            r2b2_sb = robias_t[:, 7:8]
            rob12_t = st.tile([TARGET, 2], f32, tag="rob12")
            nc.sync.dma_start(rob12_t[:], d_rob12[:])
            r1b3_sb = rob12_t[:, 0:1]
            r2b3_sb = rob12_t[:, 1:2]


            for _rep in range(nreps):
                # NOTE: the reference's per-step node_mask multiply is
                # dropped: masked inputs guarantee no edges touch virtual
                # nodes, their per-node GRU lanes never mix into real nodes,
                # and the readout re-applies mask_row.

                # ---- message passing loop ----
                def hT_chunk(s, g, hc):
                    if s == 0:
                        assert hc == 0
                        return hT0_sb[:, g, :]
                    return hT_fb[g // GPB][:, g % GPB,
                                           hc * 128:(hc + 1) * 128]

                pending_htr = []  # delayed h-transposes (PE in-order relief)

                def h_transpose(s, g):
                    # h^T for next step / readout; one graph behind the GRU
                    # so the PE never head-of-line blocks on g's update chain
                    tp2 = ps.tile([128, H], f32r, tag="pMT", bufs=2,
                                  name=f"tp2_{s}_{g}")
                    for c in range(HC):
                        nc.tensor.transpose(tp2[:, c * 128:(c + 1) * 128],
                                            h_all[:, g, c * 128:(c + 1) * 128],
                                            ident_sb[:])
                    hTg = hT_fb[g // GPB][:, g % GPB, :]
                    if g % 2 == 0:
                        nc.vector.tensor_copy(hTg, tp2[:])
                    else:
                        nc.scalar.copy(hTg, tp2[:])
                    if s < NSTEP - 1:  # fp8 h^T for next step's gh GEMM
                        # (gpsimd cannot read PSUM -> source the SBUF copy)
                        nc.gpsimd.tensor_scalar_mul(hT8_all[:, g, :, :],
                                                    hTg, 1.0)

                # ---- readout (layer-major over 4 independent chains) ----
                out_sb = st.tile([TARGET, G], f32, tag="out_sb")
                nfree = GPB * N  # 512
                r1_ws = [[r1w0_sb[:, kc, :] for kc in range(5)],
                         [r1w1_sb[:, oc, :] for oc in range(2)],
                         [r1w2_sb[:, kc, :] for kc in range(2)],
                         r1w3_sb[:]]
                r1_bs = [r1b0_sb[:],
                         [r1b1_sb[:, oc:oc + 1] for oc in range(2)],
                         r1b2_sb[:]]
                r2_ws = [[r2w0_sb[:, kc, :] for kc in range(4)],
                         [r2w1_sb[:, oc, :] for oc in range(2)],
                         [r2w2_sb[:, kc, :] for kc in range(2)],
                         r2w3_sb[:]]
                r2_bs = [r2b0_sb[:],
                         [r2b1_sb[:, oc:oc + 1] for oc in range(2)],
                         r2b2_sb[:]]
                chains = []
                for fb in range(FB):
                    gsl = slice(fb * GPB, (fb + 1) * GPB)
                    h_in_chunks = [hT_fb[fb][:, :, kc * 128:(kc + 1) * 128]
                                   for kc in range(HC)]
                    chains.append(dict(fb=fb, w="g", ws=r1_ws, bs=r1_bs,
                                       ins=h_in_chunks + [hT0_sb[:, gsl, :]]))
                    chains.append(dict(fb=fb, w="v", ws=r2_ws, bs=r2_bs,
                                       ins=h_in_chunks))
                relueng = [None, nc.vector, None, nc.vector]

                def relu_from(dst, src, bias, ci):
                    eng = relueng[ci]
                    if eng is None:
                        nc.scalar.activation(dst, src, AF.Relu, bias=bias)
                    else:
                        eng.tensor_scalar(dst, src, bias, 0.0,
                                          op0=ALU.add, op1=ALU.max)

                def ro_l0(ch, ci):
                    key = f"{ch['w']}{ch['fb']}"
                    p = ps.tile([128, nfree], f32,
                                tag="pP" if ci % 2 == 0 else "pG2", bufs=2,
                                name=f"rop0_{key}")
                    for i, (wap, rhs) in enumerate(zip(ch["ws"][0], ch["ins"])):
                        nc.tensor.matmul(p[:], wap, rhs, start=(i == 0),
                                         stop=(i == len(ch["ins"]) - 1))
                    a1 = wk.tile([128, nfree], f32r, tag="P", bufs=8,
                                 name=f"roa1_{key}")
                    relu_from(a1[:], p[:], ch["bs"][0], ci)
                    ch["a1"] = a1

                # fb=0 L0 first: it only needs graphs 0-3, so the PE can run
                # it while graph 7's update chain is still in flight; the
                # last h-transpose is flushed in between
                def ro_l1(fb):
                    for ci0, ch in enumerate(chains[2 * fb:2 * fb + 2]):
                        ci = 2 * fb + ci0
                        key = f"{ch['w']}{ch['fb']}"
                        ch["a2"] = []
                        for oc in range(2):
                            p2 = ps.tile([128, nfree], f32,
                                         tag="pP" if oc == 0 else "pG2",
                                         bufs=2, name=f"rop1_{key}_{oc}")
                            nc.tensor.matmul(p2[:], ch["ws"][1][oc],
                                             ch["a1"][:],
                                             start=True, stop=True)
                            t = wk.tile([128, nfree], f32r, tag="P", bufs=8,
                                        name=f"roa2_{key}_{oc}")
                            relu_from(t[:], p2[:], ch["bs"][1][oc],
                                      (ci + oc) % 2)
                            ch["a2"].append(t)

                def ro_l2(fb):
                    for ci0, ch in enumerate(chains[2 * fb:2 * fb + 2]):
                        ci = 2 * fb + ci0
                        key = f"{ch['w']}{ch['fb']}"
                        p3 = ps.tile([128, nfree], f32,
                                      tag="pP" if ci0 == 0 else "pG2",
                                      bufs=2, name=f"rop2_{key}")
                        for kc in range(2):
                            nc.tensor.matmul(p3[:], ch["ws"][2][kc],
                                             ch["a2"][kc][:],
                                             start=(kc == 0), stop=(kc == 1))
                        a3 = wk.tile([128, nfree], f32r, tag="P", bufs=8,
                                     name=f"roa3_{key}")
                        # always Act: the DVE queue backs up at the tail
                        relu_from(a3[:], p3[:], ch["bs"][2], 0)
                        ch["a3"] = a3

                def ro_l3(fb):
                    # g-chain first: its a3 relu (queued after the g-chain
                    # L2) is ready sooner; chv's relu overlaps L3(chg)
                    for ch in chains[2 * fb:2 * fb + 2]:
                        key = f"{ch['w']}{ch['fb']}"
                        p4 = ps.tile([TARGET, nfree], f32, tag="pGN", bufs=2,
                                     name=f"rop3_{key}")
                        nc.tensor.matmul(p4[:], ch["ws"][3], ch["a3"][:],
                                         start=True, stop=True)
                        ch["p4"] = p4

                def ro_finals(fb):
                    # finals: sum_v gate*val*mask per graph
                    fsl = slice(fb * nfree, (fb + 1) * nfree)
                    chg, chv = chains[2 * fb:2 * fb + 2]
                    # val*mask fused: (p4 + b) * mb, straight from PSUM
                    vm = wk.tile([TARGET, nfree], f32, tag="z", bufs=2,
                                 name=f"vm_{fb}")
                    nc.vector.scalar_tensor_tensor(
                        vm[:], chv["p4"][:], r2b3_sb[:], mb_sb[:, fsl],
                        op0=ALU.add, op1=ALU.mult)
                    gate_s = wk.tile([TARGET, nfree], f32, tag="r", bufs=2,
                                     name=f"gate_{fb}")
                    nc.scalar.activation(gate_s[:], chg["p4"][:], AF.Sigmoid,
                                         bias=r1b3_sb[:])
                    pr2 = wk.tile([TARGET, GPB, N], f32, tag="t1", bufs=2,
                                  name=f"pr2_{fb}")
                    nc.vector.scalar_tensor_tensor(
                        pr2[:], gate_s[:], 1.0, vm[:],
                        op0=ALU.mult, op1=ALU.mult)
                    nc.vector.tensor_reduce(
                        out_sb[:, fb * GPB:(fb + 1) * GPB], pr2[:],
                        axis=AX.X, op=ALU.add)

                for s in range(NSTEP):
                    hcs = [0] if s == 0 else list(range(HC))
                    # -- phase 1 (all graphs): projections + aggregation -> mT --
                    def projections(g):
                        P_sb = []
                        cpeng = [nc.vector.tensor_copy, nc.scalar.copy,
                                 nc.scalar.copy, nc.scalar.copy]
                        if s == 0:
                            # step-0 phase 1 is copy-bound on Act: rebalance
                            # one PSUM->SBUF copy onto DVE
                            cpeng[2] = nc.vector.tensor_copy
                        for l in range(L):
                            # step 0: spread projection PSUM across the pP
                            # and (GRU-idle) pG2 pools -> 4 effective bufs,
                            # so short s0 projections don't wait on copies
                            ptag = "pP" if l % 2 == 0 else "pG2"
                            pp = ps.tile([128, MSG], f32, tag=ptag, bufs=2,
                                         name=f"pp_{s}_{g}_{l}")
                            if s == 0 and g == 0 and l < 2:
                                # sourced from the packed boot DMA: the very
                                # first matmuls wait on one small transfer
                                nc.tensor.matmul(pp[:], boot_hT0g0,
                                                 boot_A0[l],
                                                 start=True, stop=True)
                                psb = wk.tile([128, MSG], f32r, tag="P",
                                              bufs=8, name=f"psb_{s}_{g}_{l}")
                                cpeng[l](psb[:], pp[:])
                                P_sb.append(psb)
                                continue
                            for i, hc in enumerate(hcs):
                                nc.tensor.matmul(pp[:], hT_chunk(s, g, hc),
                                                 A_sb[:, l, hc, :],
                                                 start=(i == 0),
                                                 stop=(i == len(hcs) - 1))
                            psb = wk.tile([128, MSG], f32r, tag="P", bufs=8,
                                          name=f"psb_{s}_{g}_{l}")
                            cpeng[l](psb[:], pp[:])
                            P_sb.append(psb)
                        return P_sb

                    def agg_m(g, P_sb):
                        mp = ps.tile([128, MSG], f32, tag="pMT", bufs=2,
                                     name=f"mp_{s}_{g}")
                        for l in range(L):
                            nc.tensor.matmul(mp[:], mask_sb[:, g, l, :], P_sb[l][:],
                                             start=(l == 0), stop=(l == L - 1))
                        m_sb = wk.tile([128, MSG], f32r, tag="m", bufs=2,
                                       name=f"m_{s}_{g}")
                        nc.vector.tensor_copy(m_sb[:], mp[:])
                        return m_sb

                    def mT_from(g, m_sb):
                        tp = ps.tile([128, MSG], f32r, tag="pMT", bufs=2,
                                     name=f"tp_{s}_{g}")
                        for c in range(MC):
                            nc.tensor.transpose(tp[:, c * 128:(c + 1) * 128],
                                                m_sb[:, c * 128:(c + 1) * 128],
                                                ident_sb[:])
                        mT_sb = wk.tile([128, MSG], f32r, tag="mT", bufs=6,
                                        name=f"mT_{s}_{g}")
                        nc.scalar.copy(mT_sb[:], tp[:])
                        return mT_sb

                    # last step runs fb1's graphs first so their h^T land
                    # early for the (fb1-first) readout
                    gorder = (list(range(G)) if s < NSTEP - 1
                              else list(range(GPB, G)) + list(range(GPB)))
                    mT_tiles = {}
                    for gp in range(G // 2):
                        g0, g1 = gorder[2 * gp], gorder[2 * gp + 1]
                        Ps0 = projections(g0)
                        if gp == 0 and pending_htr:
                            h_transpose(*pending_htr.pop())
                        Ps1 = projections(g1)
                        mT_tiles[g0] = mT_from(g0, agg_m(g0, Ps0))
                        mT_tiles[g1] = mT_from(g1, agg_m(g1, Ps1))
                    if s == 0:
                        # read mask + its 12-row broadcast; emitted here to
                        # keep it off the startup critical path (only the
                        # readout finals consume these)
                        mask_row = st.tile([1, G * N], f32r, tag="mask_row")
                        mb_sb = st.tile([TARGET, G * N], f32, tag="mb_sb")
                        for fb in range(FB):
                            fsl = slice(fb * GPB * N, (fb + 1) * GPB * N)
                            colsum = ps.tile([1, GPB * N], f32, tag="pMT",
                                             bufs=2, name=f"colsum{fb}")
                            nc.tensor.matmul(
                                colsum[:], onescol_sb[:],
                                hT0_sb[:, fb * GPB:(fb + 1) * GPB, :],
                                start=True, stop=True)
                            nc.vector.tensor_scalar(mask_row[:, fsl],
                                                    colsum[:], 0.0, None,
                                                    op0=ALU.not_equal)
                            mbp = ps.tile([TARGET, GPB * N], f32, tag="pMT",
                                          bufs=2, name=f"mbp{fb}")
                            nc.tensor.matmul(mbp[:], ones12_sb[:],
                                             mask_row[:, fsl],
                                             start=True, stop=True)
                            nc.scalar.copy(mb_sb[:, fsl], mbp[:])
                    # -- phase 2 (all graphs): GRU GEMMs + gates --
                    def gh8_pairs(g, sl):
                        # fp8 DoubleRow h @ Whh chunk-pairs (zero-pair at s=0)
                        if s == 0:
                            return [(hT08[:, g, :, :], whh8_sb[:, 0:2, sl])]
                        return [(hT8_all[:, g, c:c + 2, :],
                                 whh8_sb[:, c:c + 2, sl]) for c in (0, 2)]

                    def gate_mms(o, dr_mms, f32_mms):
                        # fp8-DR insts first: they depend only on hT8/bias
                        # rows, so the PE can start them before the mT copy
                        # for this graph has landed
                        ntot = len(f32_mms) + len(dr_mms)
                        i = 0
                        for lh, rh in dr_mms:
                            nc.tensor.matmul(o, lh, rh, start=(i == 0),
                                             stop=(i == ntot - 1),
                                             perf_mode=DR)
                            i += 1
                        for lh, rh in f32_mms:
                            nc.tensor.matmul(o, lh, rh, start=(i == 0),
                                             stop=(i == ntot - 1))
                            i += 1

                    for gi_, g in enumerate(gorder):
                        mT_sb = mT_tiles[g]
                        # 4) GRU gate GEMMs: gi/mT in f32r, gh + biases in
                        # fp8 DoubleRow; PSUM carries GSC x the true preacts
                        rzp = [ps.tile([128, H], f32, tag="pG2", bufs=2,
                                       name=f"g{'rz'[h_]}_{s}_{g}")
                               for h_ in range(2)]
                        gin = ps.tile([128, H], f32, tag="pGN", bufs=2,
                                      name=f"gin_{s}_{g}")
                        ghn = ps.tile([128, H], f32, tag="pGN", bufs=2,
                                      name=f"ghn_{s}_{g}")
                        gate_mms(
                            ghn[:],
                            gh8_pairs(g, slice(2 * H, 3 * H))
                            + [(ones8_t[:], crows8_t[:, :, 3 * H:4 * H])],
                            [])
                        for half in range(2):
                            sl = slice(half * H, (half + 1) * H)
                            gate_mms(
                                rzp[half][:],
                                gh8_pairs(g, sl)
                                + [(ones8_t[:], crows8_t[:, :, sl])],
                                [(mT_sb[:, c * 128:(c + 1) * 128],
                                  wih_sb[:, c, sl]) for c in range(MC)])
                        gate_mms(
                            gin[:],
                            [(ones8_t[:], crows8_t[:, :, 2 * H:3 * H])],
                            [(mT_sb[:, c * 128:(c + 1) * 128],
                              wih_sb[:, c, 2 * H:3 * H]) for c in range(MC)])
                        # 5) gate nonlinearities + state update as
                        # h' = (1-z)*n + z*h: z*h runs before tanh lands,
                        # so only 2 DVE ops trail the tanh
                        r_sb = wk.tile([128, H], f32, tag="r", bufs=2,
                                       name=f"r_{s}_{g}")
                        nc.scalar.activation(r_sb[:], rzp[0][:], AF.Sigmoid,
                                             scale=1.0 / GSC)
                        z_sb = wk.tile([128, H], f32, tag="z", bufs=2,
                                       name=f"z_{s}_{g}")
                        nc.scalar.activation(z_sb[:], rzp[1][:],
                                             AF.Sigmoid, scale=1.0 / GSC)
                        # u = 1-z via negated-scale sigmoid
                        u_sb = wk.tile([128, H], f32, tag="u", bufs=2,
                                       name=f"u_{s}_{g}")
                        nc.scalar.activation(u_sb[:], rzp[1][:], AF.Sigmoid,
                                             scale=-1.0 / GSC)
                        rhn = wk.tile([128, H], f32, tag="t1", bufs=2,
                                      name=f"rhn_{s}_{g}")
                        nc.vector.tensor_mul(rhn[:], r_sb[:], ghn[:])
                        zh = wk.tile([128, H], f32, tag="zh", bufs=2,
                                     name=f"zh_{s}_{g}")
                        nc.vector.tensor_mul(zh[:], z_sb[:], h_all[:, g, :])
                        npre = wk.tile([128, H], f32, tag="t2", bufs=2,
                                       name=f"npre_{s}_{g}")
                        nc.vector.tensor_add(npre[:], rhn[:], gin[:])
                        n_sb = wk.tile([128, H], f32, tag="n", bufs=2,
                                       name=f"n_{s}_{g}")
                        nc.scalar.activation(n_sb[:], npre[:], AF.Tanh,
                                             scale=1.0 / GSC)
                        un = wk.tile([128, H], f32, tag="t1", bufs=2,
                                     name=f"un_{s}_{g}")
                        nc.vector.tensor_mul(un[:], u_sb[:], n_sb[:])
                        nc.vector.tensor_add(h_all[:, g, :], un[:], zh[:])
                        pending_htr.append((s, g))
                        if s == NSTEP - 1 and gi_ == G - 1:
                            # fb1's h^T is complete (graphs 4-7 ran first +
                            # their transposes flushed): overlap its readout
                            # L0..L2 with the remaining fb0 GRU work
                            ro_l0(chains[2], 2)
                            ro_l0(chains[3], 3)
                            ro_l1(1)
                            ro_l2(1)
                        if gi_ >= 1:
                            h_transpose(*pending_htr.pop(0))

                # zippered, fb1 first (its graphs ran first in step 3):
                # fb1's L0 covers the last fb0 graph's update chain, then
                # layers alternate fbs so each fb's relu latency hides
                # under the other fb's GEMMs
                while pending_htr:
                    h_transpose(*pending_htr.pop(0))
                ro_l0(chains[0], 0)
                ro_l0(chains[1], 1)
                ro_l3(1)
                ro_l1(0)
                ro_finals(1)
                ro_l2(0)
                ro_l3(0)
                ro_finals(0)
                nc.sync.dma_start(d_out[:], out_sb[:])


    nc.compile()
    return nc


def _prep_core_inputs(core, g_, h_in, e, A, gru_Wih, gru_Whh, gru_bih,
                      gru_bhh, r1_Ws, r1_bs, r2_Ws, r2_bs):
    cs = slice(core * G, (core + 1) * G)
    f = np.float32
    h0 = np.asarray(h_in[cs], f)
    hT0 = np.ascontiguousarray(h_in[cs].transpose(2, 0, 1))  # [F, G, N]
    labels = np.arange(1, L + 1, dtype=f)
    # mask[w, g, l, v] = (e[g, v, w] == l+1)
    e_c = e[cs]  # [G, V, W]
    oh = (e_c[:, None, :, :] == labels[None, :, None, None]).astype(f)  # [G,L,V,W]
    mask = np.ascontiguousarray(oh.transpose(3, 0, 1, 2))  # [W, G, L, V]
    if MASK8_DMA:
        import ml_dtypes
        mask = mask.astype(ml_dtypes.float8_e4m3)
    boot = np.concatenate([hT0[:, 0, :], np.asarray(A[0, 0:128, :], f),
                           np.asarray(A[1, 0:128, :], f)], 1)
    return {
        "h0": np.ascontiguousarray(h0.transpose(1, 0, 2)),  # [N, G, F_IN]
        "hT0": hT0,
        "mask": mask,
        "boot": np.ascontiguousarray(boot),
    }


def _prep_shared_inputs(A, gru_Wih, gru_Whh, gru_bih, gru_bhh,
                        r1_Ws, r1_bs, r2_Ws, r2_bs):
    f = np.float32

    def chunk_rows(M, nch):  # [K, C] -> [128, nch, C] with K = nch*128
        K, C = M.shape
        assert K == nch * 128
        return np.ascontiguousarray(M.reshape(nch, 128, C).transpose(1, 0, 2))

    import ml_dtypes
    f8np = ml_dtypes.float8_e4m3
    GSC = np.float32(16.0)

    A_t = np.ascontiguousarray(
        A.reshape(L, HC, 128, MSG).transpose(2, 0, 1, 3))  # [128, L, HC, MSG]
    wih = chunk_rows(np.ascontiguousarray(gru_Wih.T) * GSC, MC)  # [128,MC,3H]
    whh8 = chunk_rows(np.ascontiguousarray(gru_Whh.T) * GSC, HC).astype(f8np)
    brz = (gru_bih + gru_bhh)[:2 * H].astype(f)
    bin_ = gru_bih[2 * H:].astype(f)
    bhn = gru_bhh[2 * H:].astype(f)
    crows8 = np.zeros((1, 2, 4 * H), f)
    crows8[0, 0, :] = np.concatenate([brz, bin_, bhn]) * GSC
    ones8 = np.ones((1, 2, N), f8np)

    # readout weights, transposed layout
    r1w0t = np.ascontiguousarray(r1_Ws[0].T)  # [2H, 128]
    r1w0 = np.zeros((128, 5, 128), f)
    for kc in range(4):
        r1w0[:, kc, :] = r1w0t[kc * 128:(kc + 1) * 128]
    r1w0[:, 4, :] = r1w0t[H:H + F_IN]  # h0 chunk (features 0:128 of h0 half)
    r1w1 = np.ascontiguousarray(r1_Ws[1].T.reshape(128, 2, 128))
    r1w2 = chunk_rows(np.ascontiguousarray(r1_Ws[2].T), 2)
    r1w3 = np.ascontiguousarray(r1_Ws[3].T)  # [128, 12]
    r2w0 = chunk_rows(np.ascontiguousarray(r2_Ws[0].T), 4)
    r2w1 = np.ascontiguousarray(r2_Ws[1].T.reshape(128, 2, 128))
    r2w2 = chunk_rows(np.ascontiguousarray(r2_Ws[2].T), 2)
    r2w3 = np.ascontiguousarray(r2_Ws[3].T)

    crows = np.concatenate([np.ones(N, f), np.ones(TARGET, f)])[None, :]
    identcol = np.concatenate([np.eye(128, dtype=f), np.ones((128, 1), f)], 1)
    rowb = np.concatenate([r1w0, r1w1, r1w2, r2w0, r2w1, r2w2], axis=1)
    row3 = np.stack([r1w3, r2w3], axis=1)
    robias = np.concatenate([
        r1_bs[0].reshape(-1, 1).astype(f),
        np.ascontiguousarray(r1_bs[1].reshape(2, 128).T),
        r1_bs[2].reshape(-1, 1).astype(f),
        r2_bs[0].reshape(-1, 1).astype(f),
        np.ascontiguousarray(r2_bs[1].reshape(2, 128).T),
        r2_bs[2].reshape(-1, 1).astype(f)], axis=1)
    rob12 = np.concatenate([r1_bs[3].reshape(-1, 1).astype(f),
                            r2_bs[3].reshape(-1, 1).astype(f)], axis=1)
    return {
        "A": A_t, "wih": wih, "whh8": np.ascontiguousarray(whh8),
        "crows": np.ascontiguousarray(crows),
        "crows8": crows8.astype(f8np),
        "ones8": ones8,
        "identcol": np.ascontiguousarray(identcol),
        "rowb": np.ascontiguousarray(rowb),
        "row3": np.ascontiguousarray(row3),
        "robias": np.ascontiguousarray(robias),
        "rob12": np.ascontiguousarray(rob12),
    }


def _get_nc(nreps=1):
    key = ("nc", nreps)
    if key not in _CACHE:
        _CACHE[key] = _build(nreps)
    return _CACHE[key]


def _run(in_maps, **kwargs):
    nc = _get_nc()
    return run_bass_kernel_spmd(nc, in_maps, core_ids=list(range(NCORES)),
                                **kwargs)


def make_in_maps(g, h_in, e, A, gru_Wih, gru_Whh, gru_bih, gru_bhh,
                 r1_W0, r1_b0, r1_W1, r1_b1, r1_W2, r1_b2, r1_W3, r1_b3,
                 r2_W0, r2_b0, r2_W1, r2_b1, r2_W2, r2_b2, r2_W3, r2_b3):
    r1_Ws, r1_bs = [r1_W0, r1_W1, r1_W2, r1_W3], [r1_b0, r1_b1, r1_b2, r1_b3]
    r2_Ws, r2_bs = [r2_W0, r2_W1, r2_W2, r2_W3], [r2_b0, r2_b1, r2_b2, r2_b3]
    arrs = {k: np.asarray(v, np.float32) for k, v in dict(
        g=g, h_in=h_in, e=e, A=A, gru_Wih=gru_Wih, gru_Whh=gru_Whh,
        gru_bih=gru_bih, gru_bhh=gru_bhh).items()}
    r1_Ws = [np.asarray(w, np.float32) for w in r1_Ws]
    r1_bs = [np.asarray(b, np.float32) for b in r1_bs]
    r2_Ws = [np.asarray(w, np.float32) for w in r2_Ws]
    r2_bs = [np.asarray(b, np.float32) for b in r2_bs]
    shared = _prep_shared_inputs(arrs["A"], arrs["gru_Wih"], arrs["gru_Whh"],
                                 arrs["gru_bih"], arrs["gru_bhh"],
                                 r1_Ws, r1_bs, r2_Ws, r2_bs)
    in_maps = []
    for core in range(NCORES):
        m = dict(shared)
        m.update(_prep_core_inputs(core, arrs["g"], arrs["h_in"], arrs["e"],
                                   arrs["A"], arrs["gru_Wih"], arrs["gru_Whh"],
                                   arrs["gru_bih"], arrs["gru_bhh"],
                                   r1_Ws, r1_bs, r2_Ws, r2_bs))
        in_maps.append(m)
    return in_maps


def kernel(**inputs):
    in_maps = make_in_maps(**inputs)
    res = _run(in_maps)
    out = np.zeros((B, TARGET), np.float32)
    for core in range(NCORES):
        out[core * G:(core + 1) * G] = res.results[core]["out"].T
    return out


if __name__ == "__main__":
    import reference
    inputs = {k: np.asarray(v) for k, v in reference.setup_inputs().items()}
    expected = np.asarray(reference.reference(**inputs))
    actual = kernel(**inputs)
    scale = np.abs(expected).max()
    err = np.abs(actual - expected).max() / scale
    print("Relative error:", err)



# revision 131
# speedup vs baseline: 1.0044x; 1.0031x over previous
"""MPNN-GGNN forward on 8 Trainium2 NeuronCores.

Data-parallel over the batch: 8 graphs per core. All weights replicated.
Per-core Bass/Tile kernel computes 4 message-passing + GRU steps and the
gated readout entirely on-chip; f32r (full fp32 bits, reduced-precision
multiplier) matmuls at full PE rate. The small-magnitude gh GEMM
(h @ Whh^T) and GRU bias rows run as fp8e4 DoubleRow matmuls (2 K-chunks
per instruction at 0.5 cycles/row = 4x f32r); the error-sensitive
message path (proj/agg/gi, preact sigma ~50) must stay f32r (measured:
fp8 there = 0.07 rel err, bf16 = 0.026, vs the 2e-2 gate).

Schedule notes (PE is in-order; head-of-line blocking is the enemy):
  - h-transposes run one graph behind their update chain
  - fp8-DR GRU matmuls are emitted before the mT-dependent f32r ones
  - last step runs fb1's graphs first; fb1's readout L0..L2 is injected
    into the final GRU iteration, and readout layers zipper between fbs
  - per-step node_mask multiply dropped (masked inputs make it redundant
    until the readout's mask_row)

Layout conventions per core (G = 8 graphs, N = 128 nodes, H = MSG = 512):
  h_all  [128(node), G, 512]  fp32   node-major hidden state
  hT_fb[fb] [128(feat), 4, 512]  h^T split per readout block so fb0's
         readout only depends on graphs 0-3
  hT8_all[128, G, HC, 128] fp8   h^T chunk-pairs for DoubleRow gh
  mask_sb[128(w),   G, L, 128(v)]    (e^T == l+1) one-hot adjacency
  matmul convention: out[i,j] = sum_k lhsT[k,i] * rhs[k,j]
"""

import numpy as np

import concourse.mybir as mybir
import concourse.tile as tile
from concourse import bacc
from concourse.bass_utils import run_bass_kernel_spmd

# problem constants (hardcoded per contract)
B, N, F_IN = 64, 128, 128
H, MSG, L = 512, 512, 4
NSTEP = 4
TARGET = 12
NCORES = 8
G = B // NCORES          # graphs per core
HC = H // 128            # h chunks
MC = MSG // 128          # msg chunks
FB = 2                   # readout free blocks (4 graphs x 128 nodes each)
GPB = G // FB

f32 = mybir.dt.float32
f32r = mybir.dt.float32r
f8 = mybir.dt.float8e4
AF = mybir.ActivationFunctionType
ALU = mybir.AluOpType
AX = mybir.AxisListType
DR = mybir.MatmulPerfMode.DoubleRow
GSC = 16.0  # gate-preact PSUM scale: wih/whh/biases premultiplied by 16
# Direct-mT aggregation is dead: hw forbids mixing f32r with 8/16-bit
# matmul inputs, f32r x f32r at free=128 pays the 4x narrow-moving-dim
# penalty, and a bf16 P loses too much precision (rel err 0.026 measured).
# The 0/1 mask still ships as fp8 (4x less serial prologue DMA) and is
# expanded to f32r on the idle Pool engine.
MASK8_DMA = True

_CACHE = {}


def _build(nreps=1):
    nc = bacc.Bacc("TRN2", target_bir_lowering=False)

    # ---- DRAM I/O ----
    # boot: hT0 for graph 0 + A[l=0..1, hc=0] packed in one early DMA
    d_boot = nc.dram_tensor("boot", [128, N + 2 * MSG], f32r,
                            kind="ExternalInput")
    d_h0 = nc.dram_tensor("h0", [N, G, F_IN], f32r, kind="ExternalInput")
    d_hT0 = nc.dram_tensor("hT0", [F_IN, G, N], f32r, kind="ExternalInput")
    d_mask = nc.dram_tensor("mask", [N, G, L, N],
                            f8 if MASK8_DMA else f32r, kind="ExternalInput")
    d_A = nc.dram_tensor("A", [128, L, HC, MSG], f32r, kind="ExternalInput")
    d_wih = nc.dram_tensor("wih", [128, MC, 3 * H], f32r, kind="ExternalInput")
    d_whh8 = nc.dram_tensor("whh8", [128, HC, 3 * H], f8, kind="ExternalInput")
    CR = N + TARGET  # packed const rows: ones|ones12
    d_crows = nc.dram_tensor("crows", [1, CR], f32r, kind="ExternalInput")
    # fp8 GRU bias rows (x16), block 1 all-zero (DoubleRow zero-pair partner)
    d_crows8 = nc.dram_tensor("crows8", [1, 2, 4 * H], f8, kind="ExternalInput")
    d_ones8 = nc.dram_tensor("ones8", [1, 2, N], f8, kind="ExternalInput")
    d_identcol = nc.dram_tensor("identcol", [128, 129], f32r, kind="ExternalInput")
    d_rowb = nc.dram_tensor("rowb", [128, 17, 128], f32r, kind="ExternalInput")
    d_row3 = nc.dram_tensor("row3", [128, 2, TARGET], f32r, kind="ExternalInput")
    d_robias = nc.dram_tensor("robias", [128, 8], f32, kind="ExternalInput")
    d_rob12 = nc.dram_tensor("rob12", [TARGET, 2], f32, kind="ExternalInput")
    d_out = nc.dram_tensor("out", [TARGET, G], f32, kind="ExternalOutput")

    with tile.TileContext(nc) as tc:
        with tc.tile_pool(name="st", bufs=1) as st, \
             tc.tile_pool(name="state", bufs=1) as stt, \
             tc.tile_pool(name="wk", bufs=2) as wk, \
             tc.tile_pool(name="ps", bufs=1, space="PSUM") as ps:

            # ---- static loads, in consumption order ----
            # first PE ops need hT0 + identcol + A[l, hc=0] + mask; then the
            # step-0 GRU needs wih/whh8/bias rows; the rest can trickle in
            boot_t = st.tile([128, N + 2 * MSG], f32r, tag="boot")
            nc.sync.dma_start(boot_t[:], d_boot[:])
            boot_hT0g0 = boot_t[:, 0:N]
            boot_A0 = [boot_t[:, N:N + MSG], boot_t[:, N + MSG:N + 2 * MSG]]
            hT0_sb = st.tile([F_IN, G, N], f32r, tag="hT0")
            A_sb = st.tile([128, L, HC, MSG], f32r, tag="A")
            nc.sync.dma_start(A_sb[:, 2, 0, :], d_A[:, 2, 0, :])
            nc.sync.dma_start(A_sb[:, 3, 0, :], d_A[:, 3, 0, :])
            nc.sync.dma_start(hT0_sb[:], d_hT0[:])
            nc.sync.dma_start(A_sb[:, 0, 0, :], d_A[:, 0, 0, :])
            nc.sync.dma_start(A_sb[:, 1, 0, :], d_A[:, 1, 0, :])
            identcol_t = st.tile([128, 129], f32r, tag="identcol")
            nc.sync.dma_start(identcol_t[:], d_identcol[:])
            ident_sb = identcol_t[:, 0:128]
            onescol_sb = identcol_t[:, 128:129]
            mask_sb = st.tile([N, G, L, N], f32r, tag="mask")
            if MASK8_DMA:
                mask8st = st.tile([N, 2, L, N], f8, tag="mask8st")
                for g_ in range(G):
                    sl8 = g_ % 2
                    nc.sync.dma_start(mask8st[:, sl8, :, :],
                                      d_mask[:, g_, :, :])
                    nc.gpsimd.tensor_scalar_mul(mask_sb[:, g_, :, :],
                                                mask8st[:, sl8, :, :], 1.0)
            else:
                for g_ in range(G):
                    nc.sync.dma_start(mask_sb[:, g_, :, :],
                                      d_mask[:, g_, :, :])
            wih_sb = st.tile([128, MC, 3 * H], f32r, tag="wih")
            for c in range(MC):
                nc.sync.dma_start(wih_sb[:, c, :], d_wih[:, c, :])
            whh8_sb = st.tile([128, HC, 3 * H], f8, tag="whh8")
            nc.sync.dma_start(whh8_sb[:], d_whh8[:])
            crows8_t = st.tile([1, 2, 4 * H], f8, tag="crows8")
            nc.sync.dma_start(crows8_t[:], d_crows8[:])
            ones8_t = st.tile([1, 2, N], f8, tag="ones8")
            nc.sync.dma_start(ones8_t[:], d_ones8[:])
            h_all = stt.tile([N, G, H], f32r, tag="h_all")
            nc.sync.dma_start(h_all[:, :, 0:F_IN], d_h0[:])
            nc.gpsimd.memset(h_all[:, :, F_IN:H].bitcast(f32), 0.0)
            hT_fb = [stt.tile([128, GPB, H], f32r, tag=f"hT_fb{fb_}",
                              name=f"hT_fb{fb_}") for fb_ in range(FB)]
            hT8_all = stt.tile([128, G, HC, 128], f8, tag="hT8_all")
            hT08 = stt.tile([128, G, 2, 128], f8, tag="hT08")
            nc.gpsimd.memset(hT08[:, :, 1, :], 0.0)
            nc.gpsimd.tensor_scalar_mul(hT08[:, :, 0, :], hT0_sb[:], 1.0)
            crows_t = st.tile([1, CR], f32r, tag="crows")
            nc.sync.dma_start(crows_t[:], d_crows[:])
            ones12_sb = crows_t[:, N:N + TARGET]
            for hc_ in range(1, HC):
                for l_ in range(L):
                    nc.sync.dma_start(A_sb[:, l_, hc_, :], d_A[:, l_, hc_, :])

            rowb_t = st.tile([128, 17, 128], f32r, tag="rowb")
            nc.sync.dma_start(rowb_t[:], d_rowb[:])
            r1w0_sb = rowb_t[:, 0:5, :]
            r1w1_sb = rowb_t[:, 5:7, :]
            r1w2_sb = rowb_t[:, 7:9, :]
            r2w0_sb = rowb_t[:, 9:13, :]
            r2w1_sb = rowb_t[:, 13:15, :]
            r2w2_sb = rowb_t[:, 15:17, :]
            row3_t = st.tile([128, 2, TARGET], f32r, tag="row3")
            nc.sync.dma_start(row3_t[:], d_row3[:])
            r1w3_sb = row3_t[:, 0, :]
            r2w3_sb = row3_t[:, 1, :]
            robias_t = st.tile([128, 8], f32, tag="robias")
            nc.sync.dma_start(robias_t[:], d_robias[:])
            r1b0_sb = robias_t[:, 0:1]
            r1b1_sb = robias_t[:, 1:3]
            r1b2_sb = robias_t[:, 3:4]
            r2b0_sb = robias_t[:, 4:5]
            r2b1_sb = robias_t[:, 5:7]
            r2b2_sb = robias_t[:, 7:8]
            rob12_t = st.tile([TARGET, 2], f32, tag="rob12")
            nc.sync.dma_start(rob12_t[:], d_rob12[:])
            r1b3_sb = rob12_t[:, 0:1]
            r2b3_sb = rob12_t[:, 1:2]


            for _rep in range(nreps):
                # NOTE: the reference's per-step node_mask multiply is
                # dropped: masked inputs guarantee no edges touch virtual
                # nodes, their per-node GRU lanes never mix into real nodes,
                # and the readout re-applies mask_row.

                # ---- message passing loop ----
                def hT_chunk(s, g, hc):
                    if s == 0:
                        assert hc == 0
                        return hT0_sb[:, g, :]
                    return hT_fb[g // GPB][:, g % GPB,
                                           hc * 128:(hc + 1) * 128]

                pending_htr = []  # delayed h-transposes (PE in-order relief)

                def h_transpose(s, g):
                    # h^T for next step / readout; one graph behind the GRU
                    # so the PE never head-of-line blocks on g's update chain
                    tp2 = ps.tile([128, H], f32r, tag="pMT", bufs=2,
                                  name=f"tp2_{s}_{g}")
                    for c in range(HC):
                        nc.tensor.transpose(tp2[:, c * 128:(c + 1) * 128],
                                            h_all[:, g, c * 128:(c + 1) * 128],
                                            ident_sb[:])
                    hTg = hT_fb[g // GPB][:, g % GPB, :]
                    if g % 2 == 0:
                        nc.vector.tensor_copy(hTg, tp2[:])
                    else:
                        nc.scalar.copy(hTg, tp2[:])
                    if s < NSTEP - 1:  # fp8 h^T for next step's gh GEMM
                        # (gpsimd cannot read PSUM -> source the SBUF copy)
                        nc.gpsimd.tensor_scalar_mul(hT8_all[:, g, :, :],
                                                    hTg, 1.0)

                # ---- readout (layer-major over 4 independent chains) ----
                out_sb = st.tile([TARGET, G], f32, tag="out_sb")
                nfree = GPB * N  # 512
                r1_ws = [[r1w0_sb[:, kc, :] for kc in range(5)],
                         [r1w1_sb[:, oc, :] for oc in range(2)],
                         [r1w2_sb[:, kc, :] for kc in range(2)],
                         r1w3_sb[:]]
                r1_bs = [r1b0_sb[:],
                         [r1b1_sb[:, oc:oc + 1] for oc in range(2)],
                         r1b2_sb[:]]
                r2_ws = [[r2w0_sb[:, kc, :] for kc in range(4)],
                         [r2w1_sb[:, oc, :] for oc in range(2)],
                         [r2w2_sb[:, kc, :] for kc in range(2)],
                         r2w3_sb[:]]
                r2_bs = [r2b0_sb[:],
                         [r2b1_sb[:, oc:oc + 1] for oc in range(2)],
                         r2b2_sb[:]]
                chains = []
                for fb in range(FB):
                    gsl = slice(fb * GPB, (fb + 1) * GPB)
                    h_in_chunks = [hT_fb[fb][:, :, kc * 128:(kc + 1) * 128]
                                   for kc in range(HC)]
                    chains.append(dict(fb=fb, w="g", ws=r1_ws, bs=r1_bs,
                                       ins=h_in_chunks + [hT0_sb[:, gsl, :]]))
                    chains.append(dict(fb=fb, w="v", ws=r2_ws, bs=r2_bs,
                                       ins=h_in_chunks))
                relueng = [None, nc.vector, None, nc.vector]

                def relu_from(dst, src, bias, ci):
                    eng = relueng[ci]
                    if eng is None:
                        nc.scalar.activation(dst, src, AF.Relu, bias=bias)
                    else:
                        eng.tensor_scalar(dst, src, bias, 0.0,
                                          op0=ALU.add, op1=ALU.max)

                def ro_l0(ch, ci):
                    key = f"{ch['w']}{ch['fb']}"
                    p = ps.tile([128, nfree], f32,
                                tag="pP" if ci % 2 == 0 else "pG2", bufs=2,
                                name=f"rop0_{key}")
                    for i, (wap, rhs) in enumerate(zip(ch["ws"][0], ch["ins"])):
                        nc.tensor.matmul(p[:], wap, rhs, start=(i == 0),
                                         stop=(i == len(ch["ins"]) - 1))
                    a1 = wk.tile([128, nfree], f32r, tag="P", bufs=8,
                                 name=f"roa1_{key}")
                    relu_from(a1[:], p[:], ch["bs"][0], ci)
                    ch["a1"] = a1

                # fb=0 L0 first: it only needs graphs 0-3, so the PE can run
                # it while graph 7's update chain is still in flight; the
                # last h-transpose is flushed in between
                def ro_l1(fb):
                    for ci0, ch in enumerate(chains[2 * fb:2 * fb + 2]):
                        ci = 2 * fb + ci0
                        key = f"{ch['w']}{ch['fb']}"
                        ch["a2"] = []
                        for oc in range(2):
                            p2 = ps.tile([128, nfree], f32,
                                         tag="pP" if oc == 0 else "pG2",
                                         bufs=2, name=f"rop1_{key}_{oc}")
                            nc.tensor.matmul(p2[:], ch["ws"][1][oc],
                                             ch["a1"][:],
                                             start=True, stop=True)
                            t = wk.tile([128, nfree], f32r, tag="P", bufs=8,
                                        name=f"roa2_{key}_{oc}")
                            relu_from(t[:], p2[:], ch["bs"][1][oc],
                                      (ci + oc) % 2)
                            ch["a2"].append(t)

                def ro_l2(fb):
                    for ci0, ch in enumerate(chains[2 * fb:2 * fb + 2]):
                        ci = 2 * fb + ci0
                        key = f"{ch['w']}{ch['fb']}"
                        p3 = ps.tile([128, nfree], f32,
                                      tag="pP" if ci0 == 0 else "pG2",
                                      bufs=2, name=f"rop2_{key}")
                        for kc in range(2):
                            nc.tensor.matmul(p3[:], ch["ws"][2][kc],
                                             ch["a2"][kc][:],
                                             start=(kc == 0), stop=(kc == 1))
                        a3 = wk.tile([128, nfree], f32r, tag="P", bufs=8,
                                     name=f"roa3_{key}")
                        # always Act: the DVE queue backs up at the tail
                        relu_from(a3[:], p3[:], ch["bs"][2], 0)
                        ch["a3"] = a3

                def ro_l3(fb):
                    # g-chain first: its a3 relu (queued after the g-chain
                    # L2) is ready sooner; chv's relu overlaps L3(chg)
                    for ch in chains[2 * fb:2 * fb + 2]:
                        key = f"{ch['w']}{ch['fb']}"
                        p4 = ps.tile([TARGET, nfree], f32, tag="pGN", bufs=2,
                                     name=f"rop3_{key}")
                        nc.tensor.matmul(p4[:], ch["ws"][3], ch["a3"][:],
                                         start=True, stop=True)
                        ch["p4"] = p4

                def ro_finals(fb):
                    # finals: sum_v gate*val*mask per graph
                    fsl = slice(fb * nfree, (fb + 1) * nfree)
                    chg, chv = chains[2 * fb:2 * fb + 2]
                    # val*mask fused: (p4 + b) * mb, straight from PSUM
                    vm = wk.tile([TARGET, nfree], f32, tag="z", bufs=2,
                                 name=f"vm_{fb}")
                    nc.vector.scalar_tensor_tensor(
                        vm[:], chv["p4"][:], r2b3_sb[:], mb_sb[:, fsl],
                        op0=ALU.add, op1=ALU.mult)
                    gate_s = wk.tile([TARGET, nfree], f32, tag="r", bufs=2,
                                     name=f"gate_{fb}")
                    nc.scalar.activation(gate_s[:], chg["p4"][:], AF.Sigmoid,
                                         bias=r1b3_sb[:])
                    for gg in range(GPB):
                        ga = fb * GPB + gg
                        sc = wk.tile([TARGET, N], f32, tag="t1", bufs=2,
                                     name=f"sc_{fb}_{gg}")
                        nc.vector.scalar_tensor_tensor(
                            sc[:], gate_s[:, gg * N:(gg + 1) * N], 1.0,
                            vm[:, gg * N:(gg + 1) * N],
                            op0=ALU.mult, op1=ALU.mult,
                            accum_out=out_sb[:, ga:ga + 1])

                for s in range(NSTEP):
                    hcs = [0] if s == 0 else list(range(HC))
                    # -- phase 1 (all graphs): projections + aggregation -> mT --
                    def projections(g):
                        P_sb = []
                        cpeng = [nc.vector.tensor_copy, nc.scalar.copy,
                                 nc.scalar.copy, nc.scalar.copy]
                        if s == 0:
                            # step-0 phase 1 is copy-bound on Act: rebalance
                            # one PSUM->SBUF copy onto DVE
                            cpeng[2] = nc.vector.tensor_copy
                        for l in range(L):
                            # step 0: spread projection PSUM across the pP
                            # and (GRU-idle) pG2 pools -> 4 effective bufs,
                            # so short s0 projections don't wait on copies
                            ptag = "pP" if l % 2 == 0 else "pG2"
                            pp = ps.tile([128, MSG], f32, tag=ptag, bufs=2,
                                         name=f"pp_{s}_{g}_{l}")
                            if s == 0 and g == 0 and l < 2:
                                # sourced from the packed boot DMA: the very
                                # first matmuls wait on one small transfer
                                nc.tensor.matmul(pp[:], boot_hT0g0,
                                                 boot_A0[l],
                                                 start=True, stop=True)
                                psb = wk.tile([128, MSG], f32r, tag="P",
                                              bufs=8, name=f"psb_{s}_{g}_{l}")
                                cpeng[l](psb[:], pp[:])
                                P_sb.append(psb)
                                continue
                            for i, hc in enumerate(hcs):
                                nc.tensor.matmul(pp[:], hT_chunk(s, g, hc),
                                                 A_sb[:, l, hc, :],
                                                 start=(i == 0),
                                                 stop=(i == len(hcs) - 1))
                            psb = wk.tile([128, MSG], f32r, tag="P", bufs=8,
                                          name=f"psb_{s}_{g}_{l}")
                            cpeng[l](psb[:], pp[:])
                            P_sb.append(psb)
                        return P_sb

                    def agg_m(g, P_sb):
                        mp = ps.tile([128, MSG], f32, tag="pMT", bufs=2,
                                     name=f"mp_{s}_{g}")
                        for l in range(L):
                            nc.tensor.matmul(mp[:], mask_sb[:, g, l, :], P_sb[l][:],
                                             start=(l == 0), stop=(l == L - 1))
                        m_sb = wk.tile([128, MSG], f32r, tag="m", bufs=2,
                                       name=f"m_{s}_{g}")
                        nc.vector.tensor_copy(m_sb[:], mp[:])
                        return m_sb

                    def mT_from(g, m_sb):
                        tp = ps.tile([128, MSG], f32r, tag="pMT", bufs=2,
                                     name=f"tp_{s}_{g}")
                        for c in range(MC):
                            nc.tensor.transpose(tp[:, c * 128:(c + 1) * 128],
                                                m_sb[:, c * 128:(c + 1) * 128],
                                                ident_sb[:])
                        mT_sb = wk.tile([128, MSG], f32r, tag="mT", bufs=6,
                                        name=f"mT_{s}_{g}")
                        nc.scalar.copy(mT_sb[:], tp[:])
                        return mT_sb

                    # last step runs fb1's graphs first so their h^T land
                    # early for the (fb1-first) readout
                    gorder = (list(range(G)) if s < NSTEP - 1
                              else list(range(GPB, G)) + list(range(GPB)))
                    mT_tiles = {}
                    for gp in range(G // 2):
                        g0, g1 = gorder[2 * gp], gorder[2 * gp + 1]
                        Ps0 = projections(g0)
                        if gp == 0 and pending_htr:
                            h_transpose(*pending_htr.pop())
                        Ps1 = projections(g1)
                        mT_tiles[g0] = mT_from(g0, agg_m(g0, Ps0))
                        mT_tiles[g1] = mT_from(g1, agg_m(g1, Ps1))
                    if s == 0:
                        # read mask + its 12-row broadcast; emitted here to
                        # keep it off the startup critical path (only the
                        # readout finals consume these)
                        mask_row = st.tile([1, G * N], f32r, tag="mask_row")
                        mb_sb = st.tile([TARGET, G * N], f32, tag="mb_sb")
                        for fb in range(FB):
                            fsl = slice(fb * GPB * N, (fb + 1) * GPB * N)
                            colsum = ps.tile([1, GPB * N], f32, tag="pMT",
                                             bufs=2, name=f"colsum{fb}")
                            nc.tensor.matmul(
                                colsum[:], onescol_sb[:],
                                hT0_sb[:, fb * GPB:(fb + 1) * GPB, :],
                                start=True, stop=True)
                            nc.vector.tensor_scalar(mask_row[:, fsl],
                                                    colsum[:], 0.0, None,
                                                    op0=ALU.not_equal)
                            mbp = ps.tile([TARGET, GPB * N], f32, tag="pMT",
                                          bufs=2, name=f"mbp{fb}")
                            nc.tensor.matmul(mbp[:], ones12_sb[:],
                                             mask_row[:, fsl],
                                             start=True, stop=True)
                            nc.scalar.copy(mb_sb[:, fsl], mbp[:])
                    # -- phase 2 (all graphs): GRU GEMMs + gates --
                    def gh8_pairs(g, sl):
                        # fp8 DoubleRow h @ Whh chunk-pairs (zero-pair at s=0)
                        if s == 0:
                            return [(hT08[:, g, :, :], whh8_sb[:, 0:2, sl])]
                        return [(hT8_all[:, g, c:c + 2, :],
                                 whh8_sb[:, c:c + 2, sl]) for c in (0, 2)]

                    def gate_mms(o, dr_mms, f32_mms):
                        # fp8-DR insts first: they depend only on hT8/bias
                        # rows, so the PE can start them before the mT copy
                        # for this graph has landed
                        ntot = len(f32_mms) + len(dr_mms)
                        i = 0
                        for lh, rh in dr_mms:
                            nc.tensor.matmul(o, lh, rh, start=(i == 0),
                                             stop=(i == ntot - 1),
                                             perf_mode=DR)
                            i += 1
                        for lh, rh in f32_mms:
                            nc.tensor.matmul(o, lh, rh, start=(i == 0),
                                             stop=(i == ntot - 1))
                            i += 1

                    for gi_, g in enumerate(gorder):
                        mT_sb = mT_tiles[g]
                        # 4) GRU gate GEMMs: gi/mT in f32r, gh + biases in
                        # fp8 DoubleRow; PSUM carries GSC x the true preacts
                        rzp = [ps.tile([128, H], f32, tag="pG2", bufs=2,
                                       name=f"g{'rz'[h_]}_{s}_{g}")
                               for h_ in range(2)]
                        gin = ps.tile([128, H], f32, tag="pGN", bufs=2,
                                      name=f"gin_{s}_{g}")
                        ghn = ps.tile([128, H], f32, tag="pGN", bufs=2,
                                      name=f"ghn_{s}_{g}")
                        gate_mms(
                            ghn[:],
                            gh8_pairs(g, slice(2 * H, 3 * H))
                            + [(ones8_t[:], crows8_t[:, :, 3 * H:4 * H])],
                            [])
                        for half in range(2):
                            sl = slice(half * H, (half + 1) * H)
                            gate_mms(
                                rzp[half][:],
                                gh8_pairs(g, sl)
                                + [(ones8_t[:], crows8_t[:, :, sl])],
                                [(mT_sb[:, c * 128:(c + 1) * 128],
                                  wih_sb[:, c, sl]) for c in range(MC)])
                        gate_mms(
                            gin[:],
                            [(ones8_t[:], crows8_t[:, :, 2 * H:3 * H])],
                            [(mT_sb[:, c * 128:(c + 1) * 128],
                              wih_sb[:, c, 2 * H:3 * H]) for c in range(MC)])
                        # 5) gate nonlinearities + state update as
                        # h' = (1-z)*n + z*h: z*h runs before tanh lands,
                        # so only 2 DVE ops trail the tanh
                        r_sb = wk.tile([128, H], f32, tag="r", bufs=2,
                                       name=f"r_{s}_{g}")
                        nc.scalar.activation(r_sb[:], rzp[0][:], AF.Sigmoid,
                                             scale=1.0 / GSC)
                        z_sb = wk.tile([128, H], f32, tag="z", bufs=2,
                                       name=f"z_{s}_{g}")
                        nc.scalar.activation(z_sb[:], rzp[1][:],
                                             AF.Sigmoid, scale=1.0 / GSC)
                        # u = 1-z via negated-scale sigmoid
                        u_sb = wk.tile([128, H], f32, tag="u", bufs=2,
                                       name=f"u_{s}_{g}")
                        nc.scalar.activation(u_sb[:], rzp[1][:], AF.Sigmoid,
                                             scale=-1.0 / GSC)
                        rhn = wk.tile([128, H], f32, tag="t1", bufs=2,
                                      name=f"rhn_{s}_{g}")
                        nc.vector.tensor_mul(rhn[:], r_sb[:], ghn[:])
                        zh = wk.tile([128, H], f32, tag="zh", bufs=2,
                                     name=f"zh_{s}_{g}")
                        nc.vector.tensor_mul(zh[:], z_sb[:], h_all[:, g, :])
                        npre = wk.tile([128, H], f32, tag="t2", bufs=2,
                                       name=f"npre_{s}_{g}")
                        nc.vector.tensor_add(npre[:], rhn[:], gin[:])
                        n_sb = wk.tile([128, H], f32, tag="n", bufs=2,
                                       name=f"n_{s}_{g}")
                        nc.scalar.activation(n_sb[:], npre[:], AF.Tanh,
                                             scale=1.0 / GSC)
                        un = wk.tile([128, H], f32, tag="t1", bufs=2,
                                     name=f"un_{s}_{g}")
                        nc.vector.tensor_mul(un[:], u_sb[:], n_sb[:])
                        nc.vector.tensor_add(h_all[:, g, :], un[:], zh[:])
                        pending_htr.append((s, g))
                        if s == NSTEP - 1 and gi_ == G - 1:
                            # fb1's h^T is complete (graphs 4-7 ran first +
                            # their transposes flushed): overlap its readout
                            # L0..L2 with the remaining fb0 GRU work
                            ro_l0(chains[2], 2)
                            ro_l0(chains[3], 3)
                            ro_l1(1)
                            ro_l2(1)
                        if gi_ >= 1:
                            h_transpose(*pending_htr.pop(0))

                # zippered, fb1 first (its graphs ran first in step 3):
                # fb1's L0 covers the last fb0 graph's update chain, then
                # layers alternate fbs so each fb's relu latency hides
                # under the other fb's GEMMs
                while pending_htr:
                    h_transpose(*pending_htr.pop(0))
                ro_l0(chains[0], 0)
                ro_l0(chains[1], 1)
                ro_l3(1)
                ro_l1(0)
                ro_finals(1)
                ro_l2(0)
                ro_l3(0)
                ro_finals(0)
                nc.sync.dma_start(d_out[:], out_sb[:])


    nc.compile()
    return nc


def _prep_core_inputs(core, g_, h_in, e, A, gru_Wih, gru_Whh, gru_bih,
                      gru_bhh, r1_Ws, r1_bs, r2_Ws, r2_bs):
    cs = slice(core * G, (core + 1) * G)
    f = np.float32
    h0 = np.asarray(h_in[cs], f)
    hT0 = np.ascontiguousarray(h_in[cs].transpose(2, 0, 1))  # [F, G, N]
    labels = np.arange(1, L + 1, dtype=f)
    # mask[w, g, l, v] = (e[g, v, w] == l+1)
    e_c = e[cs]  # [G, V, W]
    oh = (e_c[:, None, :, :] == labels[None, :, None, None]).astype(f)  # [G,L,V,W]
    mask = np.ascontiguousarray(oh.transpose(3, 0, 1, 2))  # [W, G, L, V]
    if MASK8_DMA:
        import ml_dtypes
        mask = mask.astype(ml_dtypes.float8_e4m3)
    boot = np.concatenate([hT0[:, 0, :], np.asarray(A[0, 0:128, :], f),
                           np.asarray(A[1, 0:128, :], f)], 1)
    return {
        "h0": np.ascontiguousarray(h0.transpose(1, 0, 2)),  # [N, G, F_IN]
        "hT0": hT0,
        "mask": mask,
        "boot": np.ascontiguousarray(boot),
    }


def _prep_shared_inputs(A, gru_Wih, gru_Whh, gru_bih, gru_bhh,
                        r1_Ws, r1_bs, r2_Ws, r2_bs):
    f = np.float32

    def chunk_rows(M, nch):  # [K, C] -> [128, nch, C] with K = nch*128
        K, C = M.shape
        assert K == nch * 128
        return np.ascontiguousarray(M.reshape(nch, 128, C).transpose(1, 0, 2))

    import ml_dtypes
    f8np = ml_dtypes.float8_e4m3
    GSC = np.float32(16.0)

    A_t = np.ascontiguousarray(
        A.reshape(L, HC, 128, MSG).transpose(2, 0, 1, 3))  # [128, L, HC, MSG]
    wih = chunk_rows(np.ascontiguousarray(gru_Wih.T) * GSC, MC)  # [128,MC,3H]
    whh8 = chunk_rows(np.ascontiguousarray(gru_Whh.T) * GSC, HC).astype(f8np)
    brz = (gru_bih + gru_bhh)[:2 * H].astype(f)
    bin_ = gru_bih[2 * H:].astype(f)
    bhn = gru_bhh[2 * H:].astype(f)
    crows8 = np.zeros((1, 2, 4 * H), f)
    crows8[0, 0, :] = np.concatenate([brz, bin_, bhn]) * GSC
    ones8 = np.ones((1, 2, N), f8np)

    # readout weights, transposed layout
    r1w0t = np.ascontiguousarray(r1_Ws[0].T)  # [2H, 128]
    r1w0 = np.zeros((128, 5, 128), f)
    for kc in range(4):
        r1w0[:, kc, :] = r1w0t[kc * 128:(kc + 1) * 128]
    r1w0[:, 4, :] = r1w0t[H:H + F_IN]  # h0 chunk (features 0:128 of h0 half)
    r1w1 = np.ascontiguousarray(r1_Ws[1].T.reshape(128, 2, 128))
    r1w2 = chunk_rows(np.ascontiguousarray(r1_Ws[2].T), 2)
    r1w3 = np.ascontiguousarray(r1_Ws[3].T)  # [128, 12]
    r2w0 = chunk_rows(np.ascontiguousarray(r2_Ws[0].T), 4)
    r2w1 = np.ascontiguousarray(r2_Ws[1].T.reshape(128, 2, 128))
    r2w2 = chunk_rows(np.ascontiguousarray(r2_Ws[2].T), 2)
    r2w3 = np.ascontiguousarray(r2_Ws[3].T)

    crows = np.concatenate([np.ones(N, f), np.ones(TARGET, f)])[None, :]
    identcol = np.concatenate([np.eye(128, dtype=f), np.ones((128, 1), f)], 1)
    rowb = np.concatenate([r1w0, r1w1, r1w2, r2w0, r2w1, r2w2], axis=1)
    row3 = np.stack([r1w3, r2w3], axis=1)
    robias = np.concatenate([
        r1_bs[0].reshape(-1, 1).astype(f),
        np.ascontiguousarray(r1_bs[1].reshape(2, 128).T),
        r1_bs[2].reshape(-1, 1).astype(f),
        r2_bs[0].reshape(-1, 1).astype(f),
        np.ascontiguousarray(r2_bs[1].reshape(2, 128).T),
        r2_bs[2].reshape(-1, 1).astype(f)], axis=1)
    rob12 = np.concatenate([r1_bs[3].reshape(-1, 1).astype(f),
                            r2_bs[3].reshape(-1, 1).astype(f)], axis=1)
    return {
        "A": A_t, "wih": wih, "whh8": np.ascontiguousarray(whh8),
        "crows": np.ascontiguousarray(crows),
        "crows8": crows8.astype(f8np),
        "ones8": ones8,
        "identcol": np.ascontiguousarray(identcol),
        "rowb": np.ascontiguousarray(rowb),
        "row3": np.ascontiguousarray(row3),
        "robias": np.ascontiguousarray(robias),
        "rob12": np.ascontiguousarray(rob12),
    }


def _get_nc(nreps=1):
    key = ("nc", nreps)
    if key not in _CACHE:
        _CACHE[key] = _build(nreps)
    return _CACHE[key]


def _run(in_maps, **kwargs):
    nc = _get_nc()
    return run_bass_kernel_spmd(nc, in_maps, core_ids=list(range(NCORES)),
                                **kwargs)


def make_in_maps(g, h_in, e, A, gru_Wih, gru_Whh, gru_bih, gru_bhh,
                 r1_W0, r1_b0, r1_W1, r1_b1, r1_W2, r1_b2, r1_W3, r1_b3,
                 r2_W0, r2_b0, r2_W1, r2_b1, r2_W2, r2_b2, r2_W3, r2_b3):
    r1_Ws, r1_bs = [r1_W0, r1_W1, r1_W2, r1_W3], [r1_b0, r1_b1, r1_b2, r1_b3]
    r2_Ws, r2_bs = [r2_W0, r2_W1, r2_W2, r2_W3], [r2_b0, r2_b1, r2_b2, r2_b3]
    arrs = {k: np.asarray(v, np.float32) for k, v in dict(
        g=g, h_in=h_in, e=e, A=A, gru_Wih=gru_Wih, gru_Whh=gru_Whh,
        gru_bih=gru_bih, gru_bhh=gru_bhh).items()}
    r1_Ws = [np.asarray(w, np.float32) for w in r1_Ws]
    r1_bs = [np.asarray(b, np.float32) for b in r1_bs]
    r2_Ws = [np.asarray(w, np.float32) for w in r2_Ws]
    r2_bs = [np.asarray(b, np.float32) for b in r2_bs]
    shared = _prep_shared_inputs(arrs["A"], arrs["gru_Wih"], arrs["gru_Whh"],
                                 arrs["gru_bih"], arrs["gru_bhh"],
                                 r1_Ws, r1_bs, r2_Ws, r2_bs)
    in_maps = []
    for core in range(NCORES):
        m = dict(shared)
        m.update(_prep_core_inputs(core, arrs["g"], arrs["h_in"], arrs["e"],
                                   arrs["A"], arrs["gru_Wih"], arrs["gru_Whh"],
                                   arrs["gru_bih"], arrs["gru_bhh"],
                                   r1_Ws, r1_bs, r2_Ws, r2_bs))
        in_maps.append(m)
    return in_maps


def kernel(**inputs):
    in_maps = make_in_maps(**inputs)
    res = _run(in_maps)
    out = np.zeros((B, TARGET), np.float32)
    for core in range(NCORES):
        out[core * G:(core + 1) * G] = res.results[core]["out"].T
    return out


if __name__ == "__main__":
    import reference
    inputs = {k: np.asarray(v) for k, v in reference.setup_inputs().items()}
    expected = np.asarray(reference.reference(**inputs))
    actual = kernel(**inputs)
    scale = np.abs(expected).max()
    err = np.abs(actual - expected).max() / scale
    print("Relative error:", err)

